# revision 1
# baseline (speedup 1.0000x reference)
"""Trainium2 Bass kernel for nn_EnhancedTFNLayer (RBF field projection +
diffusion + sampling + LN/linear epilogue), data-parallel over batch on 8 cores.

Low-rank field pipeline (R=128 orthonormal basis Q fitted on host from the
parameter inputs only):

  phi[n, j] = exp(-(p_n - c_j)^2 / (2 s^2))   anchor features (fp16
              split-precision K=8 matmul + Exp)
  C = Wq^T (phi^T emb)                        field coords
  4x diffusion: C' = SLQ C + QTW @ tanh(Qsub^T (C W_int) + b_int)
              (tanh evaluated on a 256-point subsampled grid; QTW is a
               host-fitted quadrature back-projection, factor DT included)
  sampled = phi (MQ C)
  x = sampled + emb ; out = LN2(LN1(x) @ (W_out + I))   [residuals folded]

All matmul operands bf16 (emb converted on host); LN stats via stt/ts
accum_out (sum) + tensor_tensor_reduce (sum of squares); PSUM evacuations
spread across DVE / Pool / Act engines.
"""
import sys
import hashlib
import numpy as np

for _p in ("/opt/trn_rl_repo", "/root/.axon_site/_ro/trn_rl_repo"):
    if _p not in sys.path:
        sys.path.insert(0, _p)

import concourse.bass as bass
import concourse.bacc as bacc
import concourse.tile as tile
from concourse import mybir

F32 = mybir.dt.float32
BF16 = mybir.dt.bfloat16
FP16 = mybir.dt.float16
ACTF = mybir.ActivationFunctionType
ALU = mybir.AluOpType
AXL = mybir.AxisListType

B, N, G, D = 16, 4096, 1024, 256
NUM_STEPS, DT, EPS = 4, 0.01, 1e-5
R = 128
SSUB = 256               # tanh-subsampled grid points
NT = N // 128            # 32 token tiles per batch
BL = 2                   # batches per core
NCORES = 8

_CACHE = {}


def _bf16(x):
    x = np.ascontiguousarray(x, np.float32)
    u = x.view(np.uint32)
    r = ((u >> 16) + ((u >> 15) & 1)).astype(np.uint32) << 16
    return r.view(np.float32)


def _fp16(x):
    return np.float16(np.asarray(x, np.float64).astype(np.float32)).astype(np.float32)


# --------------------------------------------------------------------------
# host-side operator fitting (float64; parameter inputs only)
# --------------------------------------------------------------------------
def _host_plan(sigma, alpha, grid, W_int, b_int, W_out, b_out,
               ln1_g, ln1_b, ln2_g, ln2_b):
    rng = np.random.default_rng(0)
    c0 = 1.0 - 2.0 * alpha * DT
    c1 = alpha * DT
    pg = np.linspace(0.0, 1.0, 8193)
    K = np.exp(-((pg[:, None] - grid[None, :]) ** 2) / (2 * sigma * sigma))
    nsyn = 384
    sub = rng.choice(len(pg), size=256, replace=False)
    Fsyn = K[sub].T @ rng.standard_normal((256, nsyn))
    Fsyn /= np.abs(Fsyn).max(0, keepdims=True) + 1e-30
    fscale = np.sqrt(N * sigma * np.sqrt(np.pi))
    wnorm = np.linalg.norm(W_int, axis=0)
    wcols = rng.choice(len(wnorm), size=nsyn)
    gains = fscale * wnorm[wcols] * rng.uniform(0.5, 2.0, nsyn)
    Tsyn = np.tanh(Fsyn * gains[None, :])
    Msvd = np.concatenate([K, (Tsyn * 0.1).T], axis=0)
    _, _, Vt = np.linalg.svd(Msvd, full_matrices=False)
    Q = Vt[:R]                                            # [R, G] orthonormal
    # anchors
    c = np.linspace(-0.08, 1.08, R)
    s = 2.2 * (c[1] - c[0])
    F = np.exp(-((pg[:, None] - c[None, :]) ** 2) / (2 * s * s))
    Qk = K @ Q.T
    Wq, *_ = np.linalg.lstsq(F, Qk, rcond=1e-8)           # [R, R]
    Qt = Q.T
    LQt = c0 * Qt.copy()
    LQt[1:-1] += c1 * (Qt[:-2] + Qt[2:])
    LQt[0] += c1 * (Qt[0] + Qt[1])
    LQt[-1] += c1 * (Qt[-2] + Qt[-1])
    SLQ = Q @ LQt                                         # [R, R]
    u = pg * (G - 1)
    i0 = np.clip(np.floor(u), 0, G - 2).astype(int)
    w = u - i0
    lerpQ = Qt[i0] * (1 - w)[:, None] + Qt[i0 + 1] * w[:, None]
    MQ, *_ = np.linalg.lstsq(F, lerpQ, rcond=1e-5)        # [R, R]

    # subsampled-tanh quadrature back-projection QTW [R, SSUB]
    subidx = np.unique(np.linspace(0, G - 1, SSUB).round().astype(int))
    assert len(subidx) == SSUB
    nsyn2 = 1024
    Fg = np.exp(-((grid[:, None] - grid[None, ::8]) ** 2) / (2 * sigma * sigma))
    fields = Fg @ rng.standard_normal((Fg.shape[1], nsyn2))
    fields /= np.abs(fields).max(0, keepdims=True) + 1e-30
    gains2 = fscale * wnorm[rng.choice(len(wnorm), size=nsyn2)] * \
        np.exp(rng.uniform(np.log(0.25), np.log(4.0), nsyn2))
    TG = np.tanh(fields * gains2[None, :])                # [G, nsyn2]
    target = Q @ TG
    A = TG[subidx, :]
    lam = 1e-6 * np.linalg.norm(A) ** 2 / A.shape[0]
    QTW = np.linalg.solve(A @ A.T + lam * np.eye(SSUB), A @ target.T).T

    # fp16 split-precision anchor coefficient matrix [8, R]
    # pp8 rows on device: [qh, qh, qlr, ph, ph, pl, 1, 1]
    a3 = -1.0 / (2 * s * s)
    a1 = c / (s * s)
    a2 = -c * c / (2 * s * s)
    a3h = _fp16(a3); a3l = a3 - a3h
    a1h = _fp16(a1); a1l = a1 - a1h
    a2h = _fp16(a2); a2l = a2 - a2h
    anch8 = np.stack([
        np.full(R, a3h), np.full(R, a3l), np.full(R, a3 / 2048.0),
        a1h, a1l, a1 / 4096.0,
        a2h, a2l,
    ], axis=0)

    # affine folds: enh_aff = enh*g1 + b1 ; v = enh_aff @ (W_out + I) + b_out
    Wp = ln1_g[:, None] * (W_out + np.eye(D))             # rows scaled by g1
    brow = b_out + ln1_b @ (W_out + np.eye(D))            # const row
    f32 = lambda x: np.ascontiguousarray(x, dtype=np.float32)
    f16 = lambda x: np.ascontiguousarray(x, dtype=np.float16)

    # bf16 const blob [128, W] (values pre-rounded to bf16, stored as f32 on
    # host; device tile dtype BF16 so DMA converts? no -- DMA does not convert.
    # Host passes ml_dtypes.bfloat16 array instead; see _pack_bf16.)
    qsub = Q[:, subidx]                                   # [R, SSUB]
    qtw_t = (QTW * DT).T.reshape(2, 128, R).transpose(1, 0, 2)  # [128,2,R]
    wi = W_int.reshape(2, 128, D).transpose(1, 0, 2)      # [128,2,D]
    wo = Wp.reshape(2, 128, D).transpose(1, 0, 2)         # [128,2,D]
    cb = np.concatenate([
        qsub,                                             # [:,0:256]
        qtw_t.reshape(128, 2 * R),                        # [:,256:512]
        SLQ.T, Wq, MQ.T,                                  # 512:640,640:768,768:896
        wi.reshape(128, 2 * D),                           # 896:1408
        wo.reshape(128, 2 * D),                           # 1408:1920
        np.eye(128),                                      # 1920:2048
    ], axis=1)
    # row blob (bf16) [1, 512+]: bint row | brow | ones128
    crow = np.concatenate([
        b_int.reshape(1, D), brow.reshape(1, D), np.ones((1, 128)),
    ], axis=1)
    # f32 misc blob [128, 5]: epsb | g2? b2? (ln2 affine rows go separately)
    cg = np.full((128, 1), EPS)
    # ln2 affine rows [128, 2*D] f32 (only DMA'd/used when ln2_aff)
    caff = np.concatenate([np.broadcast_to(ln2_g, (128, D)),
                           np.broadcast_to(ln2_b, (128, D))], axis=1)

    import ml_dtypes
    bfl = lambda x: np.ascontiguousarray(x, dtype=ml_dtypes.bfloat16)
    consts = {
        "anch8": f16(anch8),
        "ones16": f16(np.ones((2, N))),
        "cb": bfl(cb),
        "crow": bfl(crow),
        "cg": f32(cg),
        "caff": f32(caff),
    }
    flags = {
        "use_bint": bool(np.any(b_int != 0)),
        "use_brow": bool(np.any(np.abs(brow) > 1e-12)),
        "ln2_aff": bool(np.any(ln2_g != 1) or np.any(ln2_b != 0)),
    }
    return consts, flags


# --------------------------------------------------------------------------
# device module
# --------------------------------------------------------------------------
def _build_module(flags, repeats=1, parts=("s1", "diff", "epi")):
    import os
    SAFE = os.environ.get("SAFE", "0") == "1"
    FEATS = set(os.environ.get("FEATS", "").split(","))
    nc = bacc.Bacc(trn_type="TRN2")
    emb_d = nc.dram_tensor("emb", [BL, N, D], BF16, kind="ExternalInput")
    pp8_d = nc.dram_tensor("pp8", [BL, 8, N], FP16, kind="ExternalInput")
    const_specs = {
        "anch8": ([8, R], FP16),
        "ones16": ([2, N], FP16),
        "cb": ([128, 2048], BF16),
        "crow": ([1, 2 * D + 128], BF16),
        "cg": ([128, 1], F32),
        "caff": ([128, 2 * D], F32),
    }
    cd = {k: nc.dram_tensor(k, sh, dt, kind="ExternalInput")
          for k, (sh, dt) in const_specs.items()}
    out_d = nc.dram_tensor("out", [BL, N, D], BF16, kind="ExternalOutput")

    with tile.TileContext(nc) as tc:
        with tc.tile_pool(name="consts", bufs=1) as cp, \
             tc.tile_pool(name="emb", bufs=2) as embp, \
             tc.tile_pool(name="phi", bufs=2) as phip, \
             tc.tile_pool(name="coef", bufs=2) as coefp, \
             tc.tile_pool(name="pre", bufs=2) as prep, \
             tc.tile_pool(name="work", bufs=3) as wp, \
             tc.tile_pool(name="tiny", bufs=8) as tp, \
             tc.tile_pool(name="psB", bufs=1, space="PSUM") as psB:

            # ---- constants ----
            blob = {}
            for k, (sh, dt) in const_specs.items():
                if k == "caff" and not flags["ln2_aff"]:
                    continue
                blob[k] = cp.tile(sh, dt, tag=k, name=f"c_{k}")
                nc.sync.dma_start(blob[k][:], cd[k][tuple(slice(None) for _ in sh)])
            _cb = blob["cb"]
            ct = {
                "anch8": blob["anch8"],
                "qsub": _cb[:, 0:256],
                "qtw": _cb[:, 256:512].rearrange("p (a b) -> p a b", a=2),
                "slt": _cb[:, 512:640], "wq": _cb[:, 640:768],
                "mqt": _cb[:, 768:896],
                "wi": _cb[:, 896:1408].rearrange("p (a b) -> p a b", a=2),
                "wo": _cb[:, 1408:1920].rearrange("p (a b) -> p a b", a=2),
                "ident": _cb[:, 1920:2048],
                "bint_row": blob["crow"][:, 0:D],
                "brow": blob["crow"][:, D:2 * D],
                "ones1": blob["crow"][:, 2 * D:2 * D + 128],
                "epsb": blob["cg"][:, 0:1],
            }
            if flags["ln2_aff"]:
                ct["g2"] = blob["caff"][:, 0:D]
                ct["b2"] = blob["caff"][:, D:2 * D]

            import contextlib
            loopctx = tc.For_i(0, repeats, 1) if repeats > 1 else contextlib.nullcontext()
            with loopctx:
              st = [dict() for _ in range(BL)]

              def load_emb(b):
                  s = st[b]
                  s["emb"] = embp.tile([128, NT, D], BF16, tag="emb",
                                       name=f"emb_{b}")
                  eap = emb_d[b].rearrange("(t q) d -> q t d", q=128)
                  for k4 in range(16):
                      nc.sync.dma_start(s["emb"][:, 2 * k4:2 * (k4 + 1), :],
                                        eap[:, 2 * k4:2 * (k4 + 1), :])

              def prologue(b):
                  """pp8 rows [qh, qh, qlr, ph, ph, pl, 1, 1] host-computed."""
                  s = st[b]
                  pp8 = prep.tile([8, N], FP16, tag="pp8", name=f"pp8_{b}")
                  nc.gpsimd.dma_start(pp8[:], pp8_d[b])
                  s["pp8"] = pp8

              def stage1_init(b):
                  s = st[b]
                  phiT = phip.tile([R, 8, 512], BF16, tag="phiT", name=f"phiT_{b}")
                  phiN = phip.tile([128, NT, 128], BF16, tag="phiN",
                                   name=f"phiN_{b}")
                  s["phiT"], s["phiN"] = phiT, phiN
                  s["pCt"] = psB.tile([128, 2, 256], F32, tag="ps2", bufs=6,
                                      name=f"pC_{b}")

              def stage1_chunk(b, j):
                  s = st[b]
                  pp8, emb_sb = s["pp8"], s["emb"]
                  phiT, phiN = s["phiT"], s["phiN"]
                  pC = s["pCt"][:, 0, :]
                  if True:
                      psPhi = psB.tile([128, 2, 256], F32, tag="ps2", bufs=6,
                                       name=f"psPhi_{b}_{j}")
                      psPhiv = psPhi[:].rearrange("p a b -> p (a b)")
                      nc.tensor.matmul(psPhiv, ct["anch8"][:, :],
                                       pp8[:, 512 * j:512 * (j + 1)],
                                       start=True, stop=True)
                      nc.scalar.activation(phiT[:, j, :], psPhiv, ACTF.Exp)
                      ptT = psB.tile([128, 512], BF16, tag="psbf", bufs=2,
                                     name=f"ptT_{b}_{j}")
                      for h in range(4):
                          nc.tensor.transpose(ptT[:, 128 * h:128 * (h + 1)],
                                              phiT[:, j, 128 * h:128 * (h + 1)],
                                              ct["ident"][:, :])
                      # evac: alternate DVE / Act (Pool cannot read PSUM)
                      dst = phiN[:, 4 * j:4 * (j + 1), :].rearrange("p a b -> p (a b)")
                      if j % 2 == 0:
                          nc.vector.tensor_copy(dst, ptT[:])
                      else:
                          nc.scalar.copy(dst, ptT[:])
                      for h in range(4):
                          t = 4 * j + h
                          nc.tensor.matmul(pC, phiN[:, t, :], emb_sb[:, t, :],
                                           start=(t == 0), stop=(t == NT - 1))

              def stage1_fin(b):
                  s = st[b]
                  pC = s["pCt"][:, 0, :]
                  craw = coefp.tile([R, D], BF16, tag="craw", name=f"craw_{b}")
                  nc.scalar.copy(craw[:], pC)
                  pC2t = psB.tile([128, 2, 256], F32, tag="ps2", bufs=6,
                                  name=f"pC2_{b}")
                  pC2 = pC2t[:, 0, :]
                  nc.tensor.matmul(pC2, ct["wq"][:, :], craw[:],
                                   start=True, stop=True)
                  C = coefp.tile([R, D], BF16, tag="C", bufs=4, name=f"C_{b}")
                  nc.vector.tensor_copy(C[:], pC2)
                  s["C"] = C

              def diffuse_step(b, step):
                  s = st[b]
                  C = s["C"]
                  ptC = psB.tile([128, 512], BF16, tag="psbf", bufs=2,
                                 name=f"ptC_{b}_{step}")
                  for h in range(2):
                      nc.tensor.transpose(ptC[:, 128 * h:128 * (h + 1)],
                                          C[:, 128 * h:128 * (h + 1)],
                                          ct["ident"][:, :])
                  Ct = wp.tile([128, 2, 128], BF16, tag="Ct", name=f"Ct_{b}_{step}")
                  nc.vector.tensor_copy(
                      Ct[:].rearrange("p a b -> p (a b)"), ptC[:, 0:256])
                  pCWt = psB.tile([128, 2, 256], F32, tag="ps2", bufs=6,
                                  name=f"pCW_{b}_{step}")
                  pCW = pCWt[:, 0, :]
                  for h in range(2):
                      nc.tensor.matmul(pCW, Ct[:, h, :], ct["wi"][:, h, :],
                                       start=(h == 0), stop=(h == 1))
                  CWb = wp.tile([R, D], BF16, tag="CWb", name=f"CWb_{b}_{step}")
                  nc.scalar.copy(CWb[:], pCW)
                  psF = psB.tile([128, 2, 256], F32, tag="ps2", bufs=6,
                                 name=f"psF_{b}_{step}")
                  for sc in range(2):
                      nc.tensor.matmul(psF[:, sc, :],
                                       ct["qsub"][:, 128 * sc:128 * (sc + 1)],
                                       CWb[:], start=True,
                                       stop=not flags["use_bint"])
                      if flags["use_bint"]:
                          nc.tensor.matmul(psF[:, sc, :], ct["ones1"][0:1, :],
                                           ct["bint_row"][0:1, :],
                                           start=False, stop=True)
                  T = wp.tile([128, 2, 256], BF16, tag="T", name=f"T_{b}_{step}")
                  nc.scalar.activation(T[:].rearrange("p a b -> p (a b)"),
                                       psF[:].rearrange("p a b -> p (a b)"),
                                       ACTF.Tanh)
                  pCnt = psB.tile([128, 2, 256], F32, tag="ps2", bufs=6,
                                  name=f"pCn_{b}_{step}")
                  pCn = pCnt[:, 0, :]
                  nc.tensor.matmul(pCn, ct["slt"][:, :], C[:],
                                   start=True, stop=False)
                  for sc in range(2):
                      nc.tensor.matmul(pCn, ct["qtw"][:, sc, :], T[:, sc, :],
                                       start=False, stop=(sc == 1))
                  C2 = coefp.tile([R, D], BF16, tag="C", bufs=4,
                                  name=f"C_{b}_{step}")
                  nc.vector.tensor_copy(C2[:], pCn)
                  s["C"] = C2

              def finish_coef(b):
                  s = st[b]
                  pMCt = psB.tile([128, 2, 256], F32, tag="ps2", bufs=6,
                                  name=f"pMC_{b}")
                  pMC = pMCt[:, 0, :]
                  nc.tensor.matmul(pMC, ct["mqt"][:, :], s["C"][:],
                                   start=True, stop=True)
                  MC = coefp.tile([R, D], BF16, tag="MC", name=f"MC_{b}")
                  nc.scalar.copy(MC[:], pMC)
                  s["MC"] = MC

              def ep_a(b, p):
                  """psamp matmuls -> psX"""
                  s = st[b]
                  phiT, MC, emb_sb = s["phiT"], s["MC"], s["emb"]
                  e = s.setdefault("ep", {}).setdefault(p, {})
                  psX = psB.tile([128, 2, 256], F32, tag="ps2", bufs=6,
                                 name=f"psX_{b}_{p}")
                  e["psX"] = psX
                  for tp in range(2):
                      t = 2 * p + tp
                      jc, h = divmod(t, 4)
                      embadd = not SAFE
                      nc.tensor.matmul(psX[:, tp, :],
                                       phiT[:, jc, 128 * h:128 * (h + 1)],
                                       MC[:], start=True, stop=not embadd)
                      if embadd:
                          nc.tensor.matmul(psX[:, tp, :], ct["ident"][:, :],
                                           emb_sb[:, t, :], start=False,
                                           stop=True)

              def ep_b(b, p):
                  """x evac + row sums"""
                  s = st[b]
                  emb_sb = s["emb"]
                  e = s["ep"][p]
                  psX = e["psX"]
                  x_bf = wp.tile([128, 2, 256], BF16, tag="x", bufs=8,
                                 name=f"x_{b}_{p}")
                  sx = tp_.tile([128, 4], F32, tag="sx", name=f"sx_{b}_{p}")
                  e["x_bf"], e["sx"] = x_bf, sx
                  if SAFE:
                      for tp in range(2):
                          t = 2 * p + tp
                          nc.vector.scalar_tensor_tensor(
                              x_bf[:, tp, :], psX[:, tp, :], 1.0,
                              emb_sb[:, t, :], op0=ALU.mult, op1=ALU.add,
                              accum_out=sx[:, tp:tp + 1])
                      return
                  for tp in range(2):
                      nc.scalar.activation(x_bf[:, tp, :], psX[:, tp, :],
                                           ACTF.Identity,
                                           accum_out=sx[:, tp:tp + 1])

              def ep_c(b, p):
                  """LN1 stats"""
                  s = st[b]
                  e = s["ep"][p]
                  x_bf, sx = e["x_bf"], e["sx"]
                  invD = 1.0 / D
                  mv = tp_.tile([128, 4], F32, tag="mv", name=f"mv_{b}_{p}")
                  e["mv"] = mv
                  junk = wp.tile([128, 2, 256], BF16, tag="junk", bufs=4,
                                 name=f"junk_{b}_{p}")
                  for tp in range(2):
                      nc.vector.affine_mul_reduce(
                          junk[:, tp, :], sx[:, 2 + tp:3 + tp],
                          x_bf[:, tp, :], x_bf[:, tp, :], 1.0, 0.0)
                  nc.vector.tensor_scalar(mv[:, 0:2], sx[:, 0:2],
                                          invD, None, op0=ALU.mult)
                  nc.vector.tensor_mul(mv[:, 2:4], mv[:, 0:2], mv[:, 0:2])
                  rst = tp_.tile([128, 2], F32, tag="rst", name=f"rst_{b}_{p}")
                  e["rst1"] = rst
                  nc.vector.scalar_tensor_tensor(
                      mv[:, 2:4], sx[:, 2:4], invD,
                      mv[:, 2:4], op0=ALU.mult, op1=ALU.subtract)
                  nc.scalar.activation(rst[:, 0:2], mv[:, 2:4],
                                       ACTF.Sqrt, bias=ct["epsb"][:, :])
                  nc.vector.reciprocal(rst[:], rst[:])

              def ep_d1(b, p):
                  """normalize + transpose -> ptE"""
                  s = st[b]
                  e = s["ep"][p]
                  x_bf, mv, rst = e["x_bf"], e["mv"], e["rst1"]
                  enh = wp.tile([128, 2, 256], BF16, tag="enh", bufs=6,
                                name=f"enh_{b}_{p}")
                  ptE = psB.tile([128, 512], BF16, tag="psbf", bufs=2,
                                 name=f"ptE_{b}_{p}")
                  e["ptE"] = ptE
                  for tp in range(2):
                      nc.gpsimd.tensor_scalar(enh[:, tp, :], x_bf[:, tp, :],
                                              mv[:, tp:tp + 1], rst[:, tp:tp + 1],
                                              op0=ALU.subtract, op1=ALU.mult)
                      for h2 in range(2):
                          nc.tensor.transpose(
                              ptE[:, 256 * tp + 128 * h2:256 * tp + 128 * (h2 + 1)],
                              enh[:, tp, 128 * h2:128 * (h2 + 1)],
                              ct["ident"][:, :])

              def ep_d2(b, p):
                  """evac enhT + v matmul -> psV"""
                  s = st[b]
                  e = s["ep"][p]
                  ptE = e.pop("ptE")
                  enhT = wp.tile([128, 4, 128], BF16, tag="enhT", bufs=6,
                                 name=f"enhT_{b}_{p}")
                  if (not SAFE) and p % 2 == 1:
                      nc.scalar.copy(enhT[:].rearrange("p a b -> p (a b)"), ptE[:])
                  else:
                      nc.vector.tensor_copy(enhT[:].rearrange("p a b -> p (a b)"),
                                            ptE[:])
                  psV = psB.tile([128, 2, 256], F32, tag="ps2", bufs=6,
                                 name=f"psV_{b}_{p}")
                  e["psV"] = psV
                  for tp in range(2):
                      for h2 in range(2):
                          nc.tensor.matmul(psV[:, tp, :], enhT[:, 2 * tp + h2, :],
                                           ct["wo"][:, h2, :],
                                           start=(h2 == 0),
                                           stop=(h2 == 1 and not flags["use_brow"]))
                      if flags["use_brow"]:
                          nc.tensor.matmul(psV[:, tp, :], ct["ones1"][0:1, :],
                                           ct["brow"][0:1, :],
                                           start=False, stop=True)

              def ep_e(b, p):
                  """v evac + row sums"""
                  s = st[b]
                  e = s["ep"][p]
                  psV = e["psV"]
                  v_bf = wp.tile([128, 2, 256], BF16, tag="v", bufs=8,
                                 name=f"v_{b}_{p}")
                  sv = tp_.tile([128, 4], F32, tag="sv", name=f"sv_{b}_{p}")
                  e["v_bf"], e["sv"] = v_bf, sv
                  if SAFE:
                      for tp in range(2):
                          nc.vector.tensor_scalar(v_bf[:, tp, :], psV[:, tp, :],
                                                  1.0, 0.0, op0=ALU.mult,
                                                  op1=ALU.add,
                                                  accum_out=sv[:, tp:tp + 1])
                      return
                  nc.scalar.activation(v_bf[:, 0, :], psV[:, 0, :],
                                       ACTF.Identity, accum_out=sv[:, 0:1])
                  nc.vector.tensor_scalar(v_bf[:, 1, :], psV[:, 1, :],
                                          1.0, 0.0, op0=ALU.mult, op1=ALU.add,
                                          accum_out=sv[:, 1:2])

              def ep_f(b, p):
                  """LN2 stats"""
                  s = st[b]
                  e = s["ep"][p]
                  v_bf, sv = e["v_bf"], e["sv"]
                  invD = 1.0 / D
                  mv2 = tp_.tile([128, 4], F32, tag="mv2", name=f"mv2_{b}_{p}")
                  e["mv2"] = mv2
                  junk2 = wp.tile([128, 2, 256], BF16, tag="junk2", bufs=4,
                                  name=f"junk2_{b}_{p}")
                  for tp in range(2):
                      nc.vector.affine_mul_reduce(
                          junk2[:, tp, :], sv[:, 2 + tp:3 + tp],
                          v_bf[:, tp, :], v_bf[:, tp, :], 1.0, 0.0)
                  nc.vector.tensor_scalar(mv2[:, 0:2], sv[:, 0:2],
                                          invD, None, op0=ALU.mult)
                  nc.vector.tensor_mul(mv2[:, 2:4], mv2[:, 0:2], mv2[:, 0:2])
                  rst2 = tp_.tile([128, 2], F32, tag="rst2", name=f"rst2_{b}_{p}")
                  e["rst2"] = rst2
                  nc.vector.scalar_tensor_tensor(
                      mv2[:, 2:4], sv[:, 2:4], invD,
                      mv2[:, 2:4], op0=ALU.mult, op1=ALU.subtract)
                  nc.scalar.activation(rst2[:, 0:2], mv2[:, 2:4],
                                       ACTF.Sqrt, bias=ct["epsb"][:, :])
                  nc.vector.reciprocal(rst2[:], rst2[:])

              def ep_g(b, p):
                  """final normalize + DMA out"""
                  s = st[b]
                  e = s["ep"][p]
                  v_bf, mv2, rst2 = e["v_bf"], e["mv2"], e["rst2"]
                  ot = wp.tile([128, 2, 256], BF16, tag="ot", bufs=6,
                               name=f"ot_{b}_{p}")
                  for tp in range(2):
                      nc.gpsimd.tensor_scalar(ot[:, tp, :], v_bf[:, tp, :],
                                              mv2[:, tp:tp + 1],
                                              rst2[:, tp:tp + 1],
                                              op0=ALU.subtract, op1=ALU.mult)
                      if flags["ln2_aff"]:
                          nc.vector.tensor_mul(ot[:, tp, :], ot[:, tp, :],
                                               ct["g2"][:, :])
                          nc.vector.tensor_add(ot[:, tp, :], ot[:, tp, :],
                                               ct["b2"][:, :])
                  nc.sync.dma_start(
                      out_d[b].rearrange("(t q) d -> q t d", q=128)
                           [:, 2 * p:2 * (p + 1), :], ot[:])
                  s["ep"].pop(p)

              tp_ = tp
              # ---- emission: interleave the two batches at fine grain ----
              for b in range(BL):
                  prologue(b)
              for b in range(BL):
                  load_emb(b)
              if "s1" in parts:
                  for b in range(BL):
                      stage1_init(b)
                  for j in range(8):
                      for b in range(BL):
                          stage1_chunk(b, j)
                  for b in range(BL):
                      stage1_fin(b)
                  if "diff" in parts:
                      for step in range(NUM_STEPS):
                          for b in range(BL):
                              diffuse_step(b, step)
                  for b in range(BL):
                      finish_coef(b)
                  if "epi" in parts:
                      units = [(b, p) for p in range(NT // 2)
                               for b in range(BL)]
                      nu = len(units)

                      phases = (ep_a, ep_b, ep_c, ep_d1, ep_d2, ep_e,
                                ep_f, ep_g)
                      nst = len(phases)
                      for slot in range(nu + nst - 1):
                          for k, ph in enumerate(phases):
                              u = slot - k
                              if 0 <= u < nu:
                                  ph(*units[u])

    nc.compile()
    return nc


# --------------------------------------------------------------------------
# runner (same multi-core pjrt path as before)
# --------------------------------------------------------------------------
def _make_runner(nc):
    import jax
    import numpy as _np
    from jax.sharding import Mesh, PartitionSpec
    from jax.experimental.shard_map import shard_map
    from concourse import mybir as _mb
    from concourse.bass2jax import (install_neuronx_cc_hook, _bass_exec_p,
                                    partition_id_tensor)
    install_neuronx_cc_hook()
    partition_name = nc.partition_id_tensor.name if nc.partition_id_tensor else None
    in_names, out_names, out_avals, zero_outs = [], [], [], []
    for alloc in nc.m.functions[0].allocations:
        if not isinstance(alloc, _mb.MemoryLocationSet):
            continue
        name = alloc.memorylocations[0].name
        if alloc.kind == "ExternalInput":
            if name != partition_name:
                in_names.append(name)
        elif alloc.kind == "ExternalOutput":
            npdt = _mb.dt.np(alloc.dtype)
            out_names.append(name)
            out_avals.append(jax.core.ShapedArray(tuple(alloc.tensor_shape), npdt))
            zero_outs.append(_np.zeros(tuple(alloc.tensor_shape), npdt))
    n_params = len(in_names)
    n_outs = len(out_names)
    all_in = in_names + out_names + ([partition_name] if partition_name else [])

    def _body(*args):
        operands = list(args)
        if partition_name is not None:
            operands.append(partition_id_tensor())
        return tuple(_bass_exec_p.bind(
            *operands, out_avals=tuple(out_avals),
            in_names=tuple(all_in), out_names=tuple(out_names),
            lowering_input_output_aliases=(), sim_require_finite=True,
            sim_require_nnan=True, nc=nc))

    devices = jax.devices()[:NCORES]
    mesh = Mesh(_np.asarray(devices), ("core",))
    donate = tuple(range(n_params, n_params + n_outs))
    sharded = jax.jit(
        shard_map(_body, mesh=mesh,
                  in_specs=(PartitionSpec("core"),) * (n_params + n_outs),
                  out_specs=(PartitionSpec("core"),) * n_outs,
                  check_rep=False),
        donate_argnums=donate, keep_unused=True)

    def run(in_maps):
        per_core = [[_np.asarray(m[name]) for name in in_names] for m in in_maps]
        concat_in = [_np.concatenate([per_core[c][i] for c in range(NCORES)], axis=0)
                     for i in range(n_params)]
        concat_zero = [_np.zeros((NCORES * z.shape[0], *z.shape[1:]), z.dtype)
                       for z in zero_outs]
        outs = sharded(*concat_in, *concat_zero)
        outs = [_np.asarray(o) for o in outs]
        return {name: outs[i] for i, name in enumerate(out_names)}

    return run


def _host_pp8(pos):
    """pos [BL?, N] f32 -> pp8 [.., 8, N] fp16 rows [qh,qh,qlr,ph,ph,pl,1,1]"""
    p = np.asarray(pos, np.float32)
    ph = p.astype(np.float16).astype(np.float32)
    pl = ((p - ph) * 4096.0).astype(np.float16)
    qq = ph * ph
    qh = qq.astype(np.float16).astype(np.float32)
    qlr = ((qq - qh) * 2048.0 +
           ph * pl.astype(np.float32)).astype(np.float16)
    ones = np.ones_like(p, np.float16)
    return np.stack([qh.astype(np.float16), qh.astype(np.float16), qlr,
                     ph.astype(np.float16), ph.astype(np.float16), pl,
                     ones, ones], axis=-2)


def kernel(**inputs):
    import ml_dtypes
    emb = np.ascontiguousarray(inputs["embeddings"], dtype=np.float32)
    pos = np.ascontiguousarray(inputs["positions"], dtype=np.float32)
    grid = np.asarray(inputs["grid_points"], np.float64)[0, :, 0]
    params = dict(
        sigma=float(np.asarray(inputs["sigma"])),
        alpha=float(np.asarray(inputs["alpha"])),
        grid=grid,
        W_int=np.asarray(inputs["W_int"], np.float64),
        b_int=np.asarray(inputs["b_int"], np.float64),
        W_out=np.asarray(inputs["W_out"], np.float64),
        b_out=np.asarray(inputs["b_out"], np.float64),
        ln1_g=np.asarray(inputs["ln1_g"], np.float64),
        ln1_b=np.asarray(inputs["ln1_b"], np.float64),
        ln2_g=np.asarray(inputs["ln2_g"], np.float64),
        ln2_b=np.asarray(inputs["ln2_b"], np.float64),
    )
    key = hashlib.sha256(b"".join(np.asarray(v).tobytes() for v in params.values())).hexdigest()
    if key not in _CACHE:
        consts, flags = _host_plan(**params)
        nc = _build_module(flags)
        _CACHE[key] = (_make_runner(nc), consts)
    run, consts = _CACHE[key]

    embb = emb.astype(ml_dtypes.bfloat16)
    pp8 = _host_pp8(pos[..., 0])                 # [B, 8, N]
    in_maps = []
    for c in range(NCORES):
        m = {"emb": embb[BL * c:BL * (c + 1)],
             "pp8": pp8[BL * c:BL * (c + 1)]}
        m.update(consts)
        in_maps.append(m)
    outs = run(in_maps)
    return np.ascontiguousarray(outs["out"].astype(np.float32))



# revision 53
# speedup vs baseline: 1.2113x; 1.2113x over previous
"""Trainium2 Bass kernel for nn_EnhancedTFNLayer (RBF field projection +
diffusion + sampling + LN/linear epilogue), data-parallel over batch on 8 cores.

Low-rank field pipeline (R=128 orthonormal basis Q fitted on host from the
parameter inputs only):

  phi[n, j] = exp(-(p_n - c_j)^2 / (2 s^2))   anchor features (fp16
              split-precision K=8 matmul + Exp)
  C = Wq^T (phi^T emb)                        field coords
  4x diffusion: C' = SLQ C + QTW @ tanh(Qsub^T (C W_int) + b_int)
  sampledT = (MQ C)^T phi  computed D-major: psXT = MC_h^T phiT + I^T embT
  LN1 folded into the W matmul:  v_raw = x @ Wp - mu_t * wbar   (rank-1 PE
  matmul with mu transposed to a row); r_t folds into the final LN2 scale
  s_t = r * rsqrt(r^2 * var2c + eps).  LN1 stats come from 1-column PE
  matmuls (stationary xT / xT^2 pieces x ones column) in token-major form.
"""
import sys
import hashlib
import numpy as np

for _p in ("/opt/trn_rl_repo", "/root/.axon_site/_ro/trn_rl_repo"):
    if _p not in sys.path:
        sys.path.insert(0, _p)

import concourse.bass as bass
import concourse.bacc as bacc
import concourse.tile as tile
from concourse import mybir

F32 = mybir.dt.float32
BF16 = mybir.dt.bfloat16
FP16 = mybir.dt.float16
ACTF = mybir.ActivationFunctionType
ALU = mybir.AluOpType
AXL = mybir.AxisListType

B, N, G, D = 16, 4096, 1024, 256
NUM_STEPS, DT, EPS = 4, 0.01, 1e-5
R = 128
SSUB = 256               # tanh-subsampled grid points
NT = N // 128            # 32 token tiles per batch
NC = N // 512            # 8 chunks of 512 tokens per batch
BL = 2                   # batches per core
NCORES = 8
INVD = 1.0 / D

_CACHE = {}


def _fp16(x):
    return np.float16(np.asarray(x, np.float64).astype(np.float32)).astype(np.float32)


# --------------------------------------------------------------------------
# host-side operator fitting (float64; parameter inputs only)
# --------------------------------------------------------------------------
def _host_plan(sigma, alpha, grid, W_int, b_int, W_out, b_out,
               ln1_g, ln1_b, ln2_g, ln2_b):
    rng = np.random.default_rng(0)
    c0 = 1.0 - 2.0 * alpha * DT
    c1 = alpha * DT
    pg = np.linspace(0.0, 1.0, 8193)
    K = np.exp(-((pg[:, None] - grid[None, :]) ** 2) / (2 * sigma * sigma))
    nsyn = 384
    sub = rng.choice(len(pg), size=256, replace=False)
    Fsyn = K[sub].T @ rng.standard_normal((256, nsyn))
    Fsyn /= np.abs(Fsyn).max(0, keepdims=True) + 1e-30
    fscale = np.sqrt(N * sigma * np.sqrt(np.pi))
    wnorm = np.linalg.norm(W_int, axis=0)
    wcols = rng.choice(len(wnorm), size=nsyn)
    gains = fscale * wnorm[wcols] * rng.uniform(0.5, 2.0, nsyn)
    Tsyn = np.tanh(Fsyn * gains[None, :])
    Msvd = np.concatenate([K, (Tsyn * 0.1).T], axis=0)
    _, _, Vt = np.linalg.svd(Msvd, full_matrices=False)
    Q = Vt[:R]                                            # [R, G] orthonormal
    # anchors
    c = np.linspace(-0.08, 1.08, R)
    s = 2.2 * (c[1] - c[0])
    F = np.exp(-((pg[:, None] - c[None, :]) ** 2) / (2 * s * s))
    Qk = K @ Q.T
    Wq, *_ = np.linalg.lstsq(F, Qk, rcond=1e-8)           # [R, R]
    Qt = Q.T
    LQt = c0 * Qt.copy()
    LQt[1:-1] += c1 * (Qt[:-2] + Qt[2:])
    LQt[0] += c1 * (Qt[0] + Qt[1])
    LQt[-1] += c1 * (Qt[-2] + Qt[-1])
    SLQ = Q @ LQt                                         # [R, R]
    u = pg * (G - 1)
    i0 = np.clip(np.floor(u), 0, G - 2).astype(int)
    w = u - i0
    lerpQ = Qt[i0] * (1 - w)[:, None] + Qt[i0 + 1] * w[:, None]
    MQ, *_ = np.linalg.lstsq(F, lerpQ, rcond=1e-5)        # [R, R]

    # subsampled-tanh quadrature back-projection QTW [R, SSUB]
    subidx = np.unique(np.linspace(0, G - 1, SSUB).round().astype(int))
    assert len(subidx) == SSUB
    nsyn2 = 1024
    Fg = np.exp(-((grid[:, None] - grid[None, ::8]) ** 2) / (2 * sigma * sigma))
    fields = Fg @ rng.standard_normal((Fg.shape[1], nsyn2))
    fields /= np.abs(fields).max(0, keepdims=True) + 1e-30
    gains2 = fscale * wnorm[rng.choice(len(wnorm), size=nsyn2)] * \
        np.exp(rng.uniform(np.log(0.25), np.log(4.0), nsyn2))
    TG = np.tanh(fields * gains2[None, :])                # [G, nsyn2]
    target = Q @ TG
    A = TG[subidx, :]
    lam = 1e-6 * np.linalg.norm(A) ** 2 / A.shape[0]
    QTW = np.linalg.solve(A @ A.T + lam * np.eye(SSUB), A @ target.T).T

    # fp16 split-precision anchor coefficient matrix [8, R]
    # pp8 rows on device: [qh, qh, qlr, ph, ph, pl, 1, 1]
    a3 = -1.0 / (2 * s * s)
    a1 = c / (s * s)
    a2 = -c * c / (2 * s * s)
    a3h = _fp16(a3); a3l = a3 - a3h
    a1h = _fp16(a1); a1l = a1 - a1h
    a2h = _fp16(a2); a2l = a2 - a2h
    anch8 = np.stack([
        np.full(R, a3h), np.full(R, a3l), np.full(R, a3 / 2048.0),
        a1h, a1l, a1 / 4096.0,
        a2h, a2l,
    ], axis=0)

    # affine folds: enh_aff = enh*g1 + b1 ; v = enh_aff @ (W_out + I) + b_out
    Wp = ln1_g[:, None] * (W_out + np.eye(D))             # rows scaled by g1
    brow = b_out + ln1_b @ (W_out + np.eye(D))            # const row
    wbar = Wp.sum(axis=0)                                 # column sums [D]
    f32 = lambda x: np.ascontiguousarray(x, dtype=np.float32)
    f16 = lambda x: np.ascontiguousarray(x, dtype=np.float16)

    qsub = Q[:, subidx]                                   # [R, SSUB]
    qtw_t = (QTW * DT).T.reshape(2, 128, R).transpose(1, 0, 2)  # [128,2,R]
    wi = W_int.reshape(2, 128, D).transpose(1, 0, 2)      # [128,2,D]
    wo = Wp.reshape(2, 128, D).transpose(1, 0, 2)         # [128,2,D]
    onescol = np.ones((128, 1))
    cb = np.concatenate([
        qsub,                                             # [:,0:256]
        qtw_t.reshape(128, 2 * R),                        # [:,256:512]
        SLQ.T, Wq, MQ.T,                                  # 512:640,640:768,768:896
        wi.reshape(128, 2 * D),                           # 896:1408
        wo.reshape(128, 2 * D),                           # 1408:1920
        np.eye(128),                                      # 1920:2048
        onescol,                                          # 2048:2049
    ], axis=1)
    # row blob (bf16) [1, 896]: bint row | brow | ones128 | -wbar
    crow = np.concatenate([
        b_int.reshape(1, D), brow.reshape(1, D), np.ones((1, 128)),
        -wbar.reshape(1, D),
    ], axis=1)
    cg = np.concatenate([np.full((128, 1), EPS), np.eye(128)], axis=1)
    caff = np.concatenate([np.broadcast_to(ln2_g, (128, D)),
                           np.broadcast_to(ln2_b, (128, D))], axis=1)

    import ml_dtypes
    bfl = lambda x: np.ascontiguousarray(x, dtype=ml_dtypes.bfloat16)
    consts = {
        "anch8": f16(anch8),
        "cb": bfl(cb),
        "crow": bfl(crow),
        "cg": f32(cg),
        "caff": f32(caff),
    }
    flags = {
        "use_bint": bool(np.any(b_int != 0)),
        "use_brow": bool(np.any(np.abs(brow) > 1e-12)),
        "ln2_aff": bool(np.any(ln2_g != 1) or np.any(ln2_b != 0)),
    }
    return consts, flags


# --------------------------------------------------------------------------
# device module
# --------------------------------------------------------------------------
def _build_module(flags, repeats=1, parts=("s1", "diff", "epi")):
    nc = bacc.Bacc(trn_type="TRN2")
    emb_d = nc.dram_tensor("emb", [BL, N, D], BF16, kind="ExternalInput")
    embt_d = nc.dram_tensor("embT", [BL, 2, 128, N], BF16, kind="ExternalInput")
    pp8_d = nc.dram_tensor("pp8", [BL, 8, N], FP16, kind="ExternalInput")
    const_specs = {
        "anch8": ([8, R], FP16),
        "cb": ([128, 2049], BF16),
        "crow": ([1, 3 * D + 128], BF16),
        "cg": ([128, 129], F32),
        "caff": ([128, 2 * D], F32),
    }
    cd = {k: nc.dram_tensor(k, sh, dt, kind="ExternalInput")
          for k, (sh, dt) in const_specs.items()}
    out_d = nc.dram_tensor("out", [BL, N, D], BF16, kind="ExternalOutput")

    with tile.TileContext(nc) as tc:
        with tc.tile_pool(name="consts", bufs=1) as cp, \
             tc.tile_pool(name="emb", bufs=2) as embp, \
             tc.tile_pool(name="phi", bufs=2) as phip, \
             tc.tile_pool(name="coef", bufs=2) as coefp, \
             tc.tile_pool(name="pre", bufs=2) as prep, \
             tc.tile_pool(name="work", bufs=3) as wp, \
             tc.tile_pool(name="tiny", bufs=8) as tp, \
             tc.tile_pool(name="psB", bufs=1, space="PSUM") as psB:

            # ---- constants (tiles allocated here; DMAs emitted in the
            # priority order interleaved with input loads below) ----
            blob = {}
            for k, (sh, dt) in const_specs.items():
                if k == "caff" and not flags["ln2_aff"]:
                    continue
                blob[k] = cp.tile(sh, dt, tag=k, name=f"c_{k}")

            def load_const(k):
                sh = const_specs[k][0]
                nc.sync.dma_start(blob[k][:], cd[k][tuple(slice(None) for _ in sh)])

            _cb = blob["cb"]
            ct = {
                "anch8": blob["anch8"],
                "qsub": _cb[:, 0:256],
                "qtw": _cb[:, 256:512].rearrange("p (a b) -> p a b", a=2),
                "slt": _cb[:, 512:640], "wq": _cb[:, 640:768],
                "mqt": _cb[:, 768:896],
                "wi": _cb[:, 896:1408].rearrange("p (a b) -> p a b", a=2),
                "wo": _cb[:, 1408:1920].rearrange("p (a b) -> p a b", a=2),
                "ident": _cb[:, 1920:2048],
                "onescol": _cb[:, 2048:2049],
                "bint_row": blob["crow"][:, 0:D],
                "brow": blob["crow"][:, D:2 * D],
                "ones1": blob["crow"][:, 2 * D:2 * D + 128],
                "wbarneg": blob["crow"][:, 2 * D + 128:3 * D + 128],
                "epsb": blob["cg"][:, 0:1],
                "identf": blob["cg"][:, 1:129],
            }
            if flags["ln2_aff"]:
                ct["g2"] = blob["caff"][:, 0:D]
                ct["b2"] = blob["caff"][:, D:2 * D]

            import contextlib
            loopctx = tc.For_i(0, repeats, 1) if repeats > 1 else contextlib.nullcontext()
            with loopctx:
              st = [dict() for _ in range(BL)]

              def load_emb(b):
                  s = st[b]
                  s["emb"] = embp.tile([128, NT, D], BF16, tag="emb",
                                       name=f"emb_{b}")
                  eap = emb_d[b].rearrange("(t q) d -> q t d", q=128)
                  for k4 in range(4):
                      nc.sync.dma_start(s["emb"][:, 8 * k4:8 * (k4 + 1), :],
                                        eap[:, 8 * k4:8 * (k4 + 1), :])

              def load_embt(b):
                  # same SP queue, emitted after the emb loads so the shared
                  # DMA FIFO serves stage1's inputs first
                  s = st[b]
                  s["embT"] = embp.tile([128, 2, N], BF16, tag="embT",
                                        name=f"embT_{b}")
                  etap = embt_d[b].rearrange("h q t -> q h t")
                  for h in range(2):
                      nc.sync.dma_start(s["embT"][:, h, :], etap[:, h, :])

              def prologue(b):
                  """pp8 rows [qh, qh, qlr, ph, ph, pl, 1, 1] host-computed."""
                  s = st[b]
                  pp8 = prep.tile([8, N], FP16, tag="pp8", name=f"pp8_{b}")
                  nc.sync.dma_start(pp8[:], pp8_d[b])
                  s["pp8"] = pp8

              def stage1_init(b):
                  s = st[b]
                  phiT = phip.tile([R, 8, 512], BF16, tag="phiT", name=f"phiT_{b}")
                  phiN = phip.tile([128, NT, 128], BF16, tag="phiN",
                                   name=f"phiN_{b}")
                  s["phiT"], s["phiN"] = phiT, phiN
                  s["pCt"] = psB.tile([128, 2, 256], F32, tag="ps2", bufs=6,
                                      name=f"pC_{b}")

              def s1a(b, j):
                  """psPhi matmul + Exp"""
                  s = st[b]
                  pp8, phiT = s["pp8"], s["phiT"]
                  psPhi = psB.tile([128, 2, 256], F32, tag="ps2", bufs=6,
                                   name=f"psPhi_{b}_{j}")
                  psPhiv = psPhi[:].rearrange("p a b -> p (a b)")
                  nc.tensor.matmul(psPhiv, ct["anch8"][:, :],
                                   pp8[:, 512 * j:512 * (j + 1)],
                                   start=True, stop=True)
                  nc.scalar.activation(phiT[:, j, :], psPhiv, ACTF.Exp)

              def s1b(b, j):
                  """phiT transposes + evac to phiN"""
                  s = st[b]
                  phiT, phiN = s["phiT"], s["phiN"]
                  ptT = psB.tile([128, 512], BF16, tag="psbf", bufs=2,
                                 name=f"ptT_{b}_{j}")
                  for h in range(4):
                      nc.tensor.transpose(ptT[:, 128 * h:128 * (h + 1)],
                                          phiT[:, j, 128 * h:128 * (h + 1)],
                                          ct["ident"][:, :])
                  # evac on DVE (2x mode on bf16) -- Act is busy with Exp here
                  dst = phiN[:, 4 * j:4 * (j + 1), :].rearrange("p a b -> p (a b)")
                  nc.vector.tensor_copy(dst, ptT[:])

              def s1c(b, j):
                  """pC accumulation matmuls"""
                  s = st[b]
                  emb_sb, phiN = s["emb"], s["phiN"]
                  pC = s["pCt"][:, 0, :]
                  for h in range(4):
                      t = 4 * j + h
                      nc.tensor.matmul(pC, phiN[:, t, :], emb_sb[:, t, :],
                                       start=(t == 0), stop=(t == NT - 1))

              def stage1_fin(b):
                  s = st[b]
                  pC = s["pCt"][:, 0, :]
                  craw = coefp.tile([R, D], BF16, tag="craw", name=f"craw_{b}")
                  nc.scalar.copy(craw[:], pC)
                  pC2t = psB.tile([128, 2, 256], F32, tag="ps2", bufs=6,
                                  name=f"pC2_{b}")
                  pC2 = pC2t[:, 0, :]
                  nc.tensor.matmul(pC2, ct["wq"][:, :], craw[:],
                                   start=True, stop=True)
                  C = coefp.tile([R, D], BF16, tag="C", bufs=4, name=f"C_{b}")
                  nc.vector.tensor_copy(C[:], pC2)
                  s["C"] = C

              def diffuse_step(b, step):
                  s = st[b]
                  C = s["C"]
                  ptC = psB.tile([128, 512], BF16, tag="psbf", bufs=2,
                                 name=f"ptC_{b}_{step}")
                  for h in range(2):
                      nc.tensor.transpose(ptC[:, 128 * h:128 * (h + 1)],
                                          C[:, 128 * h:128 * (h + 1)],
                                          ct["ident"][:, :])
                  Ct = wp.tile([128, 2, 128], BF16, tag="Ct", name=f"Ct_{b}_{step}")
                  nc.vector.tensor_copy(
                      Ct[:].rearrange("p a b -> p (a b)"), ptC[:, 0:256])
                  pCWt = psB.tile([128, 2, 256], F32, tag="ps2", bufs=6,
                                  name=f"pCW_{b}_{step}")
                  pCW = pCWt[:, 0, :]
                  for h in range(2):
                      nc.tensor.matmul(pCW, Ct[:, h, :], ct["wi"][:, h, :],
                                       start=(h == 0), stop=(h == 1))
                  CWb = wp.tile([R, D], BF16, tag="CWb", name=f"CWb_{b}_{step}")
                  nc.scalar.copy(CWb[:], pCW)
                  psF = psB.tile([128, 2, 256], F32, tag="ps2", bufs=6,
                                 name=f"psF_{b}_{step}")
                  for sc in range(2):
                      nc.tensor.matmul(psF[:, sc, :],
                                       ct["qsub"][:, 128 * sc:128 * (sc + 1)],
                                       CWb[:], start=True,
                                       stop=not flags["use_bint"])
                      if flags["use_bint"]:
                          nc.tensor.matmul(psF[:, sc, :], ct["ones1"][0:1, :],
                                           ct["bint_row"][0:1, :],
                                           start=False, stop=True)
                  T = wp.tile([128, 2, 256], BF16, tag="T", name=f"T_{b}_{step}")
                  nc.scalar.activation(T[:].rearrange("p a b -> p (a b)"),
                                       psF[:].rearrange("p a b -> p (a b)"),
                                       ACTF.Tanh)
                  pCnt = psB.tile([128, 2, 256], F32, tag="ps2", bufs=6,
                                  name=f"pCn_{b}_{step}")
                  pCn = pCnt[:, 0, :]
                  nc.tensor.matmul(pCn, ct["slt"][:, :], C[:],
                                   start=True, stop=False)
                  for sc in range(2):
                      nc.tensor.matmul(pCn, ct["qtw"][:, sc, :], T[:, sc, :],
                                       start=False, stop=(sc == 1))
                  C2 = coefp.tile([R, D], BF16, tag="C", bufs=4,
                                  name=f"C_{b}_{step}")
                  nc.vector.tensor_copy(C2[:], pCn)
                  s["C"] = C2

              def finish_coef(b):
                  s = st[b]
                  pMCt = psB.tile([128, 2, 256], F32, tag="ps2", bufs=6,
                                  name=f"pMC_{b}")
                  pMC = pMCt[:, 0, :]
                  nc.tensor.matmul(pMC, ct["mqt"][:, :], s["C"][:],
                                   start=True, stop=True)
                  MC = coefp.tile([R, D], BF16, tag="MC", name=f"MC_{b}")
                  nc.scalar.copy(MC[:], pMC)
                  s["MC"] = MC

              # ---- epilogue v2: D-major sampled, LN1 folded into W matmul ---
              def ep_a(b, c):
                  """psXT_h = MC_h^T phiT_chunk + I^T embT_h  (PE)"""
                  s = st[b]
                  e = s.setdefault("ep", {}).setdefault(c, {})
                  phiT, MC, embT = s["phiT"], s["MC"], s["embT"]
                  e["psXT"] = []
                  for h in range(2):
                      pX = psB.tile([128, 2, 256], F32, tag="ps2", bufs=6,
                                    name=f"psXT_{b}_{c}_{h}")
                      pXv = pX[:].rearrange("p a b -> p (a b)")
                      nc.tensor.matmul(pXv, MC[:, 128 * h:128 * (h + 1)],
                                       phiT[:, c, :], start=True, stop=False)
                      nc.tensor.matmul(pXv, ct["ident"][:, :],
                                       embT[:, h, 512 * c:512 * (c + 1)],
                                       start=False, stop=True)
                      e["psXT"].append(pX)

              def ep_b(b, c):
                  """xT evac (Act)"""
                  s = st[b]
                  e = s["ep"][c]
                  xT = wp.tile([128, 2, 512], BF16, tag="xT", bufs=4,
                               name=f"xT_{b}_{c}")
                  e["xT"] = xT
                  nc.scalar.copy(xT[:, 0, :],
                                 e["psXT"][0][:].rearrange("p a b -> p (a b)"))
                  nc.vector.tensor_copy(
                      xT[:, 1, :], e["psXT"][1][:].rearrange("p a b -> p (a b)"))
                  e.pop("psXT")

              def ep_c(b, c):
                  """sq1 (DVE) + stats matmuls (PE)"""
                  s = st[b]
                  e = s["ep"][c]
                  xT = e["xT"]
                  sq = wp.tile([128, 2, 512], BF16, tag="sq", bufs=2,
                               name=f"sq_{b}_{c}")
                  nc.vector.tensor_mul(sq[:, 0, :], xT[:, 0, :], xT[:, 0, :])
                  nc.gpsimd.tensor_mul(sq[:, 1, :], xT[:, 1, :], xT[:, 1, :])
                  pS = psB.tile([128, 2, 256], F32, tag="ps2", bufs=6,
                                name=f"psS_{b}_{c}")
                  e["pS"] = pS
                  # one accumulation group at a time per 2KB zero region
                  for sub in range(4):
                      for h in range(2):
                          nc.tensor.matmul(
                              pS[:, 0, sub:sub + 1],
                              xT[:, h, 128 * sub:128 * (sub + 1)],
                              ct["onescol"][:, :],
                              start=(h == 0), stop=(h == 1))
                      for h in range(2):
                          nc.tensor.matmul(
                              pS[:, 0, 4 + sub:5 + sub],
                              sq[:, h, 128 * sub:128 * (sub + 1)],
                              ct["onescol"][:, :],
                              start=(h == 0), stop=(h == 1))

              def ep_d(b, c):
                  """LN1 stats math + mu row transpose (mu/var straight from
                  PSUM; e2 = eps*(var1+eps) folds r into the LN2 scale)"""
                  s = st[b]
                  e = s["ep"][c]
                  pS = e["pS"]
                  mu = tp.tile([128, 4], BF16, tag="mu", name=f"mu_{b}_{c}")
                  nc.scalar.activation(mu[:], pS[:, 0, 0:4], ACTF.Identity,
                                       scale=INVD)
                  var = tp.tile([128, 4], F32, tag="var", name=f"var_{b}_{c}")
                  nc.vector.tensor_mul(var[:], mu[:], mu[:])
                  nc.vector.scalar_tensor_tensor(
                      var[:], pS[:, 0, 4:8], INVD, var[:],
                      op0=ALU.mult, op1=ALU.subtract)
                  e2 = tp.tile([128, 4], F32, tag="e2", name=f"e2_{b}_{c}")
                  e["e2"] = e2
                  nc.gpsimd.tensor_scalar(e2[:], var[:], EPS, EPS * EPS,
                                          op0=ALU.mult, op1=ALU.add)
                  # transpose each mu column [128,1] -> [1,128] rows packed in
                  # the free dim (rank-1 lhsT needs base partition 0)
                  ptMu = psB.tile([128, 512], BF16, tag="psbf", bufs=2,
                                  name=f"ptMu_{b}_{c}")
                  for sub in range(4):
                      nc.tensor.transpose(ptMu[0:1, 128 * sub:128 * (sub + 1)],
                                          mu[:, sub:sub + 1],
                                          ct["ident"][:, :])
                  murow = tp.tile([1, 512], BF16, tag="murow",
                                  name=f"murow_{b}_{c}")
                  e["murow"] = murow
                  nc.vector.tensor_copy(murow[:], ptMu[0:1, 0:512])
                  e.pop("pS")

              def ep_e(b, c):
                  """psV = xT^T Wp - mu x wbar  (PE)"""
                  s = st[b]
                  e = s["ep"][c]
                  xT, murow = e["xT"], e["murow"]
                  e["psV"] = []
                  for p in range(2):
                      pV = psB.tile([128, 2, 256], F32, tag="ps2", bufs=6,
                                    name=f"psV_{b}_{c}_{p}")
                      e["psV"].append(pV)
                      for i in range(2):
                          sub = 2 * p + i
                          for h in range(2):
                              nc.tensor.matmul(
                                  pV[:, i, :],
                                  xT[:, h, 128 * sub:128 * (sub + 1)],
                                  ct["wo"][:, h, :],
                                  start=(h == 0), stop=False)
                          nc.tensor.matmul(pV[:, i, :],
                                           murow[0:1, 128 * sub:128 * (sub + 1)],
                                           ct["wbarneg"][0:1, :],
                                           start=False, stop=True)

              def ep_f(b, c):
                  """v evac + row sums (Act/DVE split)"""
                  s = st[b]
                  e = s["ep"][c]
                  v_bf = wp.tile([128, 4, 256], BF16, tag="v", bufs=4,
                                 name=f"v_{b}_{c}")
                  sv = tp.tile([128, 8], F32, tag="sv", name=f"sv_{b}_{c}")
                  e["v_bf"], e["sv"] = v_bf, sv
                  for sub in range(4):
                      pVs = e["psV"][sub // 2][:, sub % 2, :]
                      if sub % 2 == 0:
                          nc.scalar.activation(v_bf[:, sub, :], pVs,
                                               ACTF.Identity,
                                               accum_out=sv[:, sub:sub + 1])
                      else:
                          nc.vector.tensor_scalar(v_bf[:, sub, :], pVs,
                                                  1.0, 0.0, op0=ALU.mult,
                                                  op1=ALU.add,
                                                  accum_out=sv[:, sub:sub + 1])
                  e.pop("psV")

              def ep_g(b, c):
                  """sumsq2 (Pool/DVE split)"""
                  s = st[b]
                  e = s["ep"][c]
                  v_bf, sv = e["v_bf"], e["sv"]
                  junk = wp.tile([128, 4, 256], BF16, tag="junk", bufs=2,
                                 name=f"junk_{b}_{c}")
                  for sub in range(4):
                      if sub < 2:
                          nc.scalar.activation(junk[:, sub, :], v_bf[:, sub, :],
                                               ACTF.Square,
                                               accum_out=sv[:, 4 + sub:5 + sub])
                      else:
                          nc.vector.scalar_tensor_tensor(
                              junk[:, sub, :], v_bf[:, sub, :], 1.0,
                              v_bf[:, sub, :], op0=ALU.mult, op1=ALU.mult,
                              accum_out=sv[:, 4 + sub:5 + sub])

              def ep_h(b, c):
                  """LN2 stats math: s = rsqrt(var2c + e2), e2 from ep_d"""
                  s = st[b]
                  e = s["ep"][c]
                  sv, e2 = e["sv"], e["e2"]
                  negmu2 = tp.tile([128, 4], F32, tag="negmu2",
                                   name=f"negmu2_{b}_{c}")
                  e["negmu2"] = negmu2
                  nc.gpsimd.tensor_scalar(negmu2[:], sv[:, 0:4], -INVD, None,
                                          op0=ALU.mult)
                  var2 = tp.tile([128, 4], F32, tag="var2", name=f"var2_{b}_{c}")
                  nc.gpsimd.tensor_mul(var2[:], negmu2[:], negmu2[:])
                  # tm = mu2^2 - e2 ; var2cc = sv_q*invD - tm
                  nc.vector.tensor_sub(var2[:], var2[:], e2[:])
                  nc.vector.scalar_tensor_tensor(
                      var2[:], sv[:, 4:8], INVD, var2[:],
                      op0=ALU.mult, op1=ALU.subtract)
                  sfin = tp.tile([128, 4], F32, tag="sfin", name=f"sfin_{b}_{c}")
                  e["sfin"] = sfin
                  nc.scalar.activation(sfin[:], var2[:], ACTF.Sqrt)
                  nc.vector.reciprocal(sfin[:], sfin[:])
                  nms = tp.tile([128, 4], F32, tag="nms", name=f"nms_{b}_{c}")
                  e["nms"] = nms
                  nc.vector.tensor_mul(nms[:], negmu2[:], sfin[:])

              def ep_i(b, c):
                  """final normalize (DVE 4x TSP) + DMA out"""
                  s = st[b]
                  e = s["ep"][c]
                  v_bf, negmu2, sfin = e["v_bf"], e["negmu2"], e["sfin"]
                  nms = e["nms"]
                  ot = wp.tile([128, 4, 256], BF16, tag="ot", bufs=3,
                               name=f"ot_{b}_{c}")
                  for sub in range(4):
                      if sub < 2:
                          nc.scalar.activation(
                              ot[:, sub, :], v_bf[:, sub, :], ACTF.Identity,
                              scale=sfin[:, sub:sub + 1],
                              bias=nms[:, sub:sub + 1])
                      else:
                          nc.vector.tensor_scalar(
                              ot[:, sub, :], v_bf[:, sub, :],
                              negmu2[:, sub:sub + 1], sfin[:, sub:sub + 1],
                              op0=ALU.add, op1=ALU.mult)
                      if flags["ln2_aff"]:
                          nc.vector.tensor_mul(ot[:, sub, :], ot[:, sub, :],
                                               ct["g2"][:, :])
                          nc.vector.tensor_add(ot[:, sub, :], ot[:, sub, :],
                                               ct["b2"][:, :])
                  nc.sync.dma_start(
                      out_d[b].rearrange("(t q) d -> q t d", q=128)
                           [:, 4 * c:4 * (c + 1), :], ot[:])
                  s["ep"].pop(c)

              # ---- emission: staggered batches ----
              # b0 stage1 -> [b1 stage1 || b0 diffusion] -> [b0 epilogue ||
              # b1 diffusion] -> b1 epilogue.  embT loads deferred so the
              # DMA FIFO serves pp8/emb first.
              for b in range(BL):
                  prologue(b)
              load_const("anch8")
              load_const("cb")
              load_emb(0)
              load_const("crow")
              load_const("cg")
              if flags["ln2_aff"]:
                  load_const("caff")
              load_emb(1)
              if "s1" in parts:
                  for b in range(BL):
                      stage1_init(b)

                  def s1_rotate(b, sph):
                      """pipeline the given stage-1 phases over the 8 chunks"""
                      for slot in range(8 + len(sph) - 1):
                          for k, ph in enumerate(sph):
                              u = slot - k
                              if 0 <= u < 8:
                                  ph(b, u)

                  s1_rotate(0, (s1a, s1b, s1c))
                  stage1_fin(0)
                  for b in range(BL):
                      load_embt(b)
                  # b1 phi chain (needs only pp8) first; then the emb-gated
                  # pC matmuls woven between b0 diffusion steps so neither
                  # stalls the in-order PE queue for long
                  do_diff = "diff" in parts

                  def s1_weave(b):
                      sph = (s1a, s1b, s1c)
                      for slot in range(8 + len(sph) - 1):
                          if do_diff and slot % 2 == 1 and slot // 2 < NUM_STEPS:
                              diffuse_step(0, slot // 2)
                          for k, ph in enumerate(sph):
                              u = slot - k
                              if 0 <= u < 8:
                                  ph(b, u)

                  s1_weave(1)
                  stage1_fin(1)
                  finish_coef(0)
                  if "epi" in parts:
                      units = [(0, c) for c in range(NC)] + \
                              [(1, c) for c in range(NC)]
                      nu = len(units)
                      phases = (ep_a, ep_b, ep_c, ep_d, ep_e, ep_f,
                                ep_g, ep_h, ep_i)
                      nst = len(phases)
                      # weave diffusion(b1) + finish_coef(1) into the early
                      # slots (b0 epilogue units), before b1 units need MC
                      extra2 = {}
                      if do_diff:
                          for sp in range(NUM_STEPS):
                              extra2[2 * sp] = [lambda s=sp: diffuse_step(1, s)]
                      extra2.setdefault(NUM_STEPS * 2 - 1, []).append(
                          lambda: finish_coef(1))
                      for slot in range(nu + nst - 1):
                          for fn in extra2.get(slot, ()):
                              fn()
                          for k, ph in enumerate(phases):
                              u = slot - k
                              if 0 <= u < nu:
                                  ph(*units[u])
                  else:
                      if do_diff:
                          for sp in range(NUM_STEPS):
                              diffuse_step(1, sp)
                      finish_coef(1)

    nc.compile()
    return nc


# --------------------------------------------------------------------------
# runner (same multi-core pjrt path as before)
# --------------------------------------------------------------------------
def _make_runner(nc):
    import jax
    import numpy as _np
    from jax.sharding import Mesh, PartitionSpec
    from jax.experimental.shard_map import shard_map
    from concourse import mybir as _mb
    from concourse.bass2jax import (install_neuronx_cc_hook, _bass_exec_p,
                                    partition_id_tensor)
    install_neuronx_cc_hook()
    partition_name = nc.partition_id_tensor.name if nc.partition_id_tensor else None
    in_names, out_names, out_avals, zero_outs = [], [], [], []
    for alloc in nc.m.functions[0].allocations:
        if not isinstance(alloc, _mb.MemoryLocationSet):
            continue
        name = alloc.memorylocations[0].name
        if alloc.kind == "ExternalInput":
            if name != partition_name:
                in_names.append(name)
        elif alloc.kind == "ExternalOutput":
            npdt = _mb.dt.np(alloc.dtype)
            out_names.append(name)
            out_avals.append(jax.core.ShapedArray(tuple(alloc.tensor_shape), npdt))
            zero_outs.append(_np.zeros(tuple(alloc.tensor_shape), npdt))
    n_params = len(in_names)
    n_outs = len(out_names)
    all_in = in_names + out_names + ([partition_name] if partition_name else [])

    def _body(*args):
        operands = list(args)
        if partition_name is not None:
            operands.append(partition_id_tensor())
        return tuple(_bass_exec_p.bind(
            *operands, out_avals=tuple(out_avals),
            in_names=tuple(all_in), out_names=tuple(out_names),
            lowering_input_output_aliases=(), sim_require_finite=True,
            sim_require_nnan=True, nc=nc))

    devices = jax.devices()[:NCORES]
    mesh = Mesh(_np.asarray(devices), ("core",))
    donate = tuple(range(n_params, n_params + n_outs))
    sharded = jax.jit(
        shard_map(_body, mesh=mesh,
                  in_specs=(PartitionSpec("core"),) * (n_params + n_outs),
                  out_specs=(PartitionSpec("core"),) * n_outs,
                  check_rep=False),
        donate_argnums=donate, keep_unused=True)

    def run(in_maps):
        per_core = [[_np.asarray(m[name]) for name in in_names] for m in in_maps]
        concat_in = [_np.concatenate([per_core[c][i] for c in range(NCORES)], axis=0)
                     for i in range(n_params)]
        concat_zero = [_np.zeros((NCORES * z.shape[0], *z.shape[1:]), z.dtype)
                       for z in zero_outs]
        outs = sharded(*concat_in, *concat_zero)
        outs = [_np.asarray(o) for o in outs]
        return {name: outs[i] for i, name in enumerate(out_names)}

    return run


def _host_pp8(pos):
    """pos [BL?, N] f32 -> pp8 [.., 8, N] fp16 rows [qh,qh,qlr,ph,ph,pl,1,1]"""
    p = np.asarray(pos, np.float32)
    ph = p.astype(np.float16).astype(np.float32)
    pl = ((p - ph) * 4096.0).astype(np.float16)
    qq = ph * ph
    qh = qq.astype(np.float16).astype(np.float32)
    qlr = ((qq - qh) * 2048.0 +
           ph * pl.astype(np.float32)).astype(np.float16)
    ones = np.ones_like(p, np.float16)
    return np.stack([qh.astype(np.float16), qh.astype(np.float16), qlr,
                     ph.astype(np.float16), ph.astype(np.float16), pl,
                     ones, ones], axis=-2)


def _host_embt(embb):
    """emb bf16 [B?, N, D] -> embT bf16 [B?, 2, 128, N]"""
    return np.ascontiguousarray(
        embb.transpose(0, 2, 1).reshape(embb.shape[0], 2, 128, N))


def kernel(**inputs):
    import ml_dtypes
    emb = np.ascontiguousarray(inputs["embeddings"], dtype=np.float32)
    pos = np.ascontiguousarray(inputs["positions"], dtype=np.float32)
    grid = np.asarray(inputs["grid_points"], np.float64)[0, :, 0]
    params = dict(
        sigma=float(np.asarray(inputs["sigma"])),
        alpha=float(np.asarray(inputs["alpha"])),
        grid=grid,
        W_int=np.asarray(inputs["W_int"], np.float64),
        b_int=np.asarray(inputs["b_int"], np.float64),
        W_out=np.asarray(inputs["W_out"], np.float64),
        b_out=np.asarray(inputs["b_out"], np.float64),
        ln1_g=np.asarray(inputs["ln1_g"], np.float64),
        ln1_b=np.asarray(inputs["ln1_b"], np.float64),
        ln2_g=np.asarray(inputs["ln2_g"], np.float64),
        ln2_b=np.asarray(inputs["ln2_b"], np.float64),
    )
    key = hashlib.sha256(b"".join(np.asarray(v).tobytes() for v in params.values())).hexdigest()
    if key not in _CACHE:
        consts, flags = _host_plan(**params)
        nc = _build_module(flags)
        _CACHE[key] = (_make_runner(nc), consts)
    run, consts = _CACHE[key]

    embb = emb.astype(ml_dtypes.bfloat16)
    embt = _host_embt(embb)
    pp8 = _host_pp8(pos[..., 0])                 # [B, 8, N]
    in_maps = []
    for c in range(NCORES):
        m = {"emb": embb[BL * c:BL * (c + 1)],
             "embT": embt[BL * c:BL * (c + 1)],
             "pp8": pp8[BL * c:BL * (c + 1)]}
        m.update(consts)
        in_maps.append(m)
    outs = run(in_maps)
    return np.ascontiguousarray(outs["out"].astype(np.float32))


# revision 60
# speedup vs baseline: 1.3021x; 1.0749x over previous
"""Trainium2 Bass kernel for nn_EnhancedTFNLayer (RBF field projection +
diffusion + sampling + LN/linear epilogue), data-parallel over batch on 8 cores.

Low-rank field pipeline (R=128 orthonormal basis Q fitted on host from the
parameter inputs only):

  phi[n, j] = exp(-(p_n - c_j)^2 / (2 s^2))   anchor features (fp16
              split-precision K=8 matmul + Exp)
  C = Wq^T (phi^T emb)                        field coords
  4x diffusion: C' = SLQ C + QTW @ tanh(Qsub^T (C W_int) + b_int)
  sampledT = (MQ C)^T phi  computed D-major: psXT = MC_h^T phiT + I^T embT
  LN1 folded into the W matmul:  v_raw = x @ Wp - mu_t * wbar   (rank-1 PE
  matmul with mu transposed to a row); r_t folds into the final LN2 scale
  s_t = r * rsqrt(r^2 * var2c + eps).  LN1 stats come from 1-column PE
  matmuls (stationary xT / xT^2 pieces x ones column) in token-major form.
"""
import sys
import hashlib
import numpy as np

for _p in ("/opt/trn_rl_repo", "/root/.axon_site/_ro/trn_rl_repo"):
    if _p not in sys.path:
        sys.path.insert(0, _p)

import concourse.bass as bass
import concourse.bacc as bacc
import concourse.tile as tile
from concourse import mybir

F32 = mybir.dt.float32
BF16 = mybir.dt.bfloat16
FP16 = mybir.dt.float16
ACTF = mybir.ActivationFunctionType
ALU = mybir.AluOpType
AXL = mybir.AxisListType

B, N, G, D = 16, 4096, 1024, 256
NUM_STEPS, DT, EPS = 4, 0.01, 1e-5
R = 128
SSUB = 256               # tanh-subsampled grid points
NT = N // 128            # 32 token tiles per batch
NC = N // 512            # 8 chunks of 512 tokens per batch
BL = 2                   # batches per core
NCORES = 8
INVD = 1.0 / D

_CACHE = {}


def _fp16(x):
    return np.float16(np.asarray(x, np.float64).astype(np.float32)).astype(np.float32)


# --------------------------------------------------------------------------
# host-side operator fitting (float64; parameter inputs only)
# --------------------------------------------------------------------------
def _host_plan(sigma, alpha, grid, W_int, b_int, W_out, b_out,
               ln1_g, ln1_b, ln2_g, ln2_b):
    rng = np.random.default_rng(0)
    c0 = 1.0 - 2.0 * alpha * DT
    c1 = alpha * DT
    pg = np.linspace(0.0, 1.0, 8193)
    K = np.exp(-((pg[:, None] - grid[None, :]) ** 2) / (2 * sigma * sigma))
    nsyn = 384
    sub = rng.choice(len(pg), size=256, replace=False)
    Fsyn = K[sub].T @ rng.standard_normal((256, nsyn))
    Fsyn /= np.abs(Fsyn).max(0, keepdims=True) + 1e-30
    fscale = np.sqrt(N * sigma * np.sqrt(np.pi))
    wnorm = np.linalg.norm(W_int, axis=0)
    wcols = rng.choice(len(wnorm), size=nsyn)
    gains = fscale * wnorm[wcols] * rng.uniform(0.5, 2.0, nsyn)
    Tsyn = np.tanh(Fsyn * gains[None, :])
    Msvd = np.concatenate([K, (Tsyn * 0.1).T], axis=0)
    _, _, Vt = np.linalg.svd(Msvd, full_matrices=False)
    Q = Vt[:R]                                            # [R, G] orthonormal
    # anchors
    c = np.linspace(-0.08, 1.08, R)
    s = 2.2 * (c[1] - c[0])
    F = np.exp(-((pg[:, None] - c[None, :]) ** 2) / (2 * s * s))
    Qk = K @ Q.T
    Wq, *_ = np.linalg.lstsq(F, Qk, rcond=1e-8)           # [R, R]
    Qt = Q.T
    LQt = c0 * Qt.copy()
    LQt[1:-1] += c1 * (Qt[:-2] + Qt[2:])
    LQt[0] += c1 * (Qt[0] + Qt[1])
    LQt[-1] += c1 * (Qt[-2] + Qt[-1])
    SLQ = Q @ LQt                                         # [R, R]
    u = pg * (G - 1)
    i0 = np.clip(np.floor(u), 0, G - 2).astype(int)
    w = u - i0
    lerpQ = Qt[i0] * (1 - w)[:, None] + Qt[i0 + 1] * w[:, None]
    MQ, *_ = np.linalg.lstsq(F, lerpQ, rcond=1e-5)        # [R, R]

    # subsampled-tanh quadrature back-projection QTW [R, SSUB]
    subidx = np.unique(np.linspace(0, G - 1, SSUB).round().astype(int))
    assert len(subidx) == SSUB
    nsyn2 = 1024
    Fg = np.exp(-((grid[:, None] - grid[None, ::8]) ** 2) / (2 * sigma * sigma))
    fields = Fg @ rng.standard_normal((Fg.shape[1], nsyn2))
    fields /= np.abs(fields).max(0, keepdims=True) + 1e-30
    gains2 = fscale * wnorm[rng.choice(len(wnorm), size=nsyn2)] * \
        np.exp(rng.uniform(np.log(0.25), np.log(4.0), nsyn2))
    TG = np.tanh(fields * gains2[None, :])                # [G, nsyn2]
    target = Q @ TG
    A = TG[subidx, :]
    lam = 1e-6 * np.linalg.norm(A) ** 2 / A.shape[0]
    QTW = np.linalg.solve(A @ A.T + lam * np.eye(SSUB), A @ target.T).T

    # fp16 split-precision anchor coefficient matrix [8, R]
    # pp8 rows on device: [qh, qh, qlr, ph, ph, pl, 1, 1]
    a3 = -1.0 / (2 * s * s)
    a1 = c / (s * s)
    a2 = -c * c / (2 * s * s)
    a3h = _fp16(a3); a3l = a3 - a3h
    a1h = _fp16(a1); a1l = a1 - a1h
    a2h = _fp16(a2); a2l = a2 - a2h
    anch8 = np.stack([
        np.full(R, a3h), np.full(R, a3l), np.full(R, a3 / 2048.0),
        a1h, a1l, a1 / 4096.0,
        a2h, a2l,
    ], axis=0)

    # affine folds: enh_aff = enh*g1 + b1 ; v = enh_aff @ (W_out + I) + b_out
    Wp = ln1_g[:, None] * (W_out + np.eye(D))             # rows scaled by g1
    brow = b_out + ln1_b @ (W_out + np.eye(D))            # const row
    wbar = Wp.sum(axis=0)                                 # column sums [D]
    f32 = lambda x: np.ascontiguousarray(x, dtype=np.float32)
    f16 = lambda x: np.ascontiguousarray(x, dtype=np.float16)

    qsub = Q[:, subidx]                                   # [R, SSUB]
    qtw_t = (QTW * DT).T.reshape(2, 128, R).transpose(1, 0, 2)  # [128,2,R]
    wi = W_int.reshape(2, 128, D).transpose(1, 0, 2)      # [128,2,D]
    wo = Wp.reshape(2, 128, D).transpose(1, 0, 2)         # [128,2,D]
    onescol = np.ones((128, 1))
    wtil = Wp.sum(axis=1).reshape(2, 128).T              # [128, 2] row sums
    cb = np.concatenate([
        qsub,                                             # [:,0:256]
        qtw_t.reshape(128, 2 * R),                        # [:,256:512]
        SLQ.T, Wq, MQ.T,                                  # 512:640,640:768,768:896
        wi.reshape(128, 2 * D),                           # 896:1408
        wo.reshape(128, 2 * D),                           # 1408:1920
        np.eye(128),                                      # 1920:2048
        onescol,                                          # 2048:2049
        wtil,                                             # 2049:2051
    ], axis=1)
    # row blob (bf16) [1, 896]: bint row | brow | ones128 | -wbar
    crow = np.concatenate([
        b_int.reshape(1, D), brow.reshape(1, D), np.ones((1, 128)),
        -wbar.reshape(1, D),
    ], axis=1)
    swD = float(wbar.sum()) * (1.0 / D)
    cg = np.concatenate([np.full((128, 1), EPS), np.eye(128),
                         np.full((128, 1), swD)], axis=1)
    caff = np.concatenate([np.broadcast_to(ln2_g, (128, D)),
                           np.broadcast_to(ln2_b, (128, D))], axis=1)

    import ml_dtypes
    bfl = lambda x: np.ascontiguousarray(x, dtype=ml_dtypes.bfloat16)
    consts = {
        "anch8": f16(anch8),
        "cb": bfl(cb),
        "crow": bfl(crow),
        "cg": f32(cg),
        "caff": f32(caff),
    }
    flags = {
        "use_bint": bool(np.any(b_int != 0)),
        "use_brow": bool(np.any(np.abs(brow) > 1e-12)),
        "ln2_aff": bool(np.any(ln2_g != 1) or np.any(ln2_b != 0)),
    }
    return consts, flags


# --------------------------------------------------------------------------
# device module
# --------------------------------------------------------------------------
def _build_module(flags, repeats=1, parts=("s1", "diff", "epi")):
    nc = bacc.Bacc(trn_type="TRN2")
    emb_d = nc.dram_tensor("emb", [BL, N, D], BF16, kind="ExternalInput")
    embt_d = nc.dram_tensor("embT", [BL, 2, 128, N], BF16, kind="ExternalInput")
    pp8_d = nc.dram_tensor("pp8", [BL, 8, N], FP16, kind="ExternalInput")
    const_specs = {
        "anch8": ([8, R], FP16),
        "cb": ([128, 2051], BF16),
        "crow": ([1, 3 * D + 128], BF16),
        "cg": ([128, 130], F32),
        "caff": ([128, 2 * D], F32),
    }
    cd = {k: nc.dram_tensor(k, sh, dt, kind="ExternalInput")
          for k, (sh, dt) in const_specs.items()}
    out_d = nc.dram_tensor("out", [BL, N, D], BF16, kind="ExternalOutput")

    with tile.TileContext(nc) as tc:
        with tc.tile_pool(name="consts", bufs=1) as cp, \
             tc.tile_pool(name="emb", bufs=2) as embp, \
             tc.tile_pool(name="phi", bufs=2) as phip, \
             tc.tile_pool(name="coef", bufs=2) as coefp, \
             tc.tile_pool(name="pre", bufs=2) as prep, \
             tc.tile_pool(name="work", bufs=3) as wp, \
             tc.tile_pool(name="tiny", bufs=8) as tp, \
             tc.tile_pool(name="psB", bufs=1, space="PSUM") as psB:

            # ---- constants (tiles allocated here; DMAs emitted in the
            # priority order interleaved with input loads below) ----
            blob = {}
            for k, (sh, dt) in const_specs.items():
                if k == "caff" and not flags["ln2_aff"]:
                    continue
                blob[k] = cp.tile(sh, dt, tag=k, name=f"c_{k}")

            def load_const(k):
                sh = const_specs[k][0]
                nc.sync.dma_start(blob[k][:], cd[k][tuple(slice(None) for _ in sh)])

            _cb = blob["cb"]
            ct = {
                "anch8": blob["anch8"],
                "qsub": _cb[:, 0:256],
                "qtw": _cb[:, 256:512].rearrange("p (a b) -> p a b", a=2),
                "slt": _cb[:, 512:640], "wq": _cb[:, 640:768],
                "mqt": _cb[:, 768:896],
                "wi": _cb[:, 896:1408].rearrange("p (a b) -> p a b", a=2),
                "wo": _cb[:, 1408:1920].rearrange("p (a b) -> p a b", a=2),
                "ident": _cb[:, 1920:2048],
                "onescol": _cb[:, 2048:2049],
                "wtil": _cb[:, 2049:2051],
                "bint_row": blob["crow"][:, 0:D],
                "brow": blob["crow"][:, D:2 * D],
                "ones1": blob["crow"][:, 2 * D:2 * D + 128],
                "wbarneg": blob["crow"][:, 2 * D + 128:3 * D + 128],
                "epsb": blob["cg"][:, 0:1],
                "identf": blob["cg"][:, 1:129],
                "swd": blob["cg"][:, 129:130],
            }
            if flags["ln2_aff"]:
                ct["g2"] = blob["caff"][:, 0:D]
                ct["b2"] = blob["caff"][:, D:2 * D]

            import contextlib
            loopctx = tc.For_i(0, repeats, 1) if repeats > 1 else contextlib.nullcontext()
            with loopctx:
              st = [dict() for _ in range(BL)]

              def load_emb(b):
                  s = st[b]
                  s["emb"] = embp.tile([128, NT, D], BF16, tag="emb",
                                       name=f"emb_{b}")
                  eap = emb_d[b].rearrange("(t q) d -> q t d", q=128)
                  for k4 in range(4):
                      nc.sync.dma_start(s["emb"][:, 8 * k4:8 * (k4 + 1), :],
                                        eap[:, 8 * k4:8 * (k4 + 1), :])

              def load_embt(b):
                  # same SP queue, emitted after the emb loads so the shared
                  # DMA FIFO serves stage1's inputs first
                  s = st[b]
                  s["embT"] = embp.tile([128, 2, N], BF16, tag="embT",
                                        name=f"embT_{b}")
                  etap = embt_d[b].rearrange("h q t -> q h t")
                  for h in range(2):
                      nc.sync.dma_start(s["embT"][:, h, :], etap[:, h, :])

              def prologue(b):
                  """pp8 rows [qh, qh, qlr, ph, ph, pl, 1, 1] host-computed."""
                  s = st[b]
                  pp8 = prep.tile([8, N], FP16, tag="pp8", name=f"pp8_{b}")
                  nc.sync.dma_start(pp8[:], pp8_d[b])
                  s["pp8"] = pp8

              def stage1_init(b):
                  s = st[b]
                  phiT = phip.tile([R, 8, 512], BF16, tag="phiT", name=f"phiT_{b}")
                  phiN = phip.tile([128, NT, 128], BF16, tag="phiN",
                                   name=f"phiN_{b}")
                  s["phiT"], s["phiN"] = phiT, phiN
                  s["pCt"] = psB.tile([128, 2, 256], F32, tag="ps2", bufs=6,
                                      name=f"pC_{b}")

              def s1a(b, j):
                  """psPhi matmul + Exp"""
                  s = st[b]
                  pp8, phiT = s["pp8"], s["phiT"]
                  psPhi = psB.tile([128, 2, 256], F32, tag="ps2", bufs=6,
                                   name=f"psPhi_{b}_{j}")
                  psPhiv = psPhi[:].rearrange("p a b -> p (a b)")
                  nc.tensor.matmul(psPhiv, ct["anch8"][:, :],
                                   pp8[:, 512 * j:512 * (j + 1)],
                                   start=True, stop=True)
                  nc.scalar.activation(phiT[:, j, :], psPhiv, ACTF.Exp)

              def s1b(b, j):
                  """phiT transposes + evac to phiN"""
                  s = st[b]
                  phiT, phiN = s["phiT"], s["phiN"]
                  ptT = psB.tile([128, 512], BF16, tag="psbf", bufs=2,
                                 name=f"ptT_{b}_{j}")
                  for h in range(4):
                      nc.tensor.transpose(ptT[:, 128 * h:128 * (h + 1)],
                                          phiT[:, j, 128 * h:128 * (h + 1)],
                                          ct["ident"][:, :])
                  # evac on DVE (2x mode on bf16) -- Act is busy with Exp here
                  dst = phiN[:, 4 * j:4 * (j + 1), :].rearrange("p a b -> p (a b)")
                  nc.vector.tensor_copy(dst, ptT[:])

              def s1c(b, j):
                  """pC accumulation matmuls"""
                  s = st[b]
                  emb_sb, phiN = s["emb"], s["phiN"]
                  pC = s["pCt"][:, 0, :]
                  for h in range(4):
                      t = 4 * j + h
                      nc.tensor.matmul(pC, phiN[:, t, :], emb_sb[:, t, :],
                                       start=(t == 0), stop=(t == NT - 1))

              def stage1_fin(b):
                  s = st[b]
                  pC = s["pCt"][:, 0, :]
                  craw = coefp.tile([R, D], BF16, tag="craw", name=f"craw_{b}")
                  nc.scalar.copy(craw[:], pC)
                  pC2t = psB.tile([128, 2, 256], F32, tag="ps2", bufs=6,
                                  name=f"pC2_{b}")
                  pC2 = pC2t[:, 0, :]
                  nc.tensor.matmul(pC2, ct["wq"][:, :], craw[:],
                                   start=True, stop=True)
                  C = coefp.tile([R, D], BF16, tag="C", bufs=4, name=f"C_{b}")
                  nc.vector.tensor_copy(C[:], pC2)
                  s["C"] = C

              def diffuse_step(b, step):
                  s = st[b]
                  C = s["C"]
                  ptC = psB.tile([128, 512], BF16, tag="psbf", bufs=2,
                                 name=f"ptC_{b}_{step}")
                  for h in range(2):
                      nc.tensor.transpose(ptC[:, 128 * h:128 * (h + 1)],
                                          C[:, 128 * h:128 * (h + 1)],
                                          ct["ident"][:, :])
                  Ct = wp.tile([128, 2, 128], BF16, tag="Ct", name=f"Ct_{b}_{step}")
                  nc.vector.tensor_copy(
                      Ct[:].rearrange("p a b -> p (a b)"), ptC[:, 0:256])
                  pCWt = psB.tile([128, 2, 256], F32, tag="ps2", bufs=6,
                                  name=f"pCW_{b}_{step}")
                  pCW = pCWt[:, 0, :]
                  for h in range(2):
                      nc.tensor.matmul(pCW, Ct[:, h, :], ct["wi"][:, h, :],
                                       start=(h == 0), stop=(h == 1))
                  CWb = wp.tile([R, D], BF16, tag="CWb", name=f"CWb_{b}_{step}")
                  nc.scalar.copy(CWb[:], pCW)
                  psF = psB.tile([128, 2, 256], F32, tag="ps2", bufs=6,
                                 name=f"psF_{b}_{step}")
                  for sc in range(2):
                      nc.tensor.matmul(psF[:, sc, :],
                                       ct["qsub"][:, 128 * sc:128 * (sc + 1)],
                                       CWb[:], start=True,
                                       stop=not flags["use_bint"])
                      if flags["use_bint"]:
                          nc.tensor.matmul(psF[:, sc, :], ct["ones1"][0:1, :],
                                           ct["bint_row"][0:1, :],
                                           start=False, stop=True)
                  T = wp.tile([128, 2, 256], BF16, tag="T", name=f"T_{b}_{step}")
                  nc.scalar.activation(T[:].rearrange("p a b -> p (a b)"),
                                       psF[:].rearrange("p a b -> p (a b)"),
                                       ACTF.Tanh)
                  pCnt = psB.tile([128, 2, 256], F32, tag="ps2", bufs=6,
                                  name=f"pCn_{b}_{step}")
                  pCn = pCnt[:, 0, :]
                  nc.tensor.matmul(pCn, ct["slt"][:, :], C[:],
                                   start=True, stop=False)
                  for sc in range(2):
                      nc.tensor.matmul(pCn, ct["qtw"][:, sc, :], T[:, sc, :],
                                       start=False, stop=(sc == 1))
                  C2 = coefp.tile([R, D], BF16, tag="C", bufs=4,
                                  name=f"C_{b}_{step}")
                  nc.vector.tensor_copy(C2[:], pCn)
                  s["C"] = C2

              def finish_coef(b):
                  s = st[b]
                  pMCt = psB.tile([128, 2, 256], F32, tag="ps2", bufs=6,
                                  name=f"pMC_{b}")
                  pMC = pMCt[:, 0, :]
                  nc.tensor.matmul(pMC, ct["mqt"][:, :], s["C"][:],
                                   start=True, stop=True)
                  MC = coefp.tile([R, D], BF16, tag="MC", name=f"MC_{b}")
                  nc.scalar.copy(MC[:], pMC)
                  s["MC"] = MC

              # ---- epilogue v2: D-major sampled, LN1 folded into W matmul ---
              def ep_a(b, c):
                  """psXT_h = MC_h^T phiT_chunk + I^T embT_h  (PE)"""
                  s = st[b]
                  e = s.setdefault("ep", {}).setdefault(c, {})
                  phiT, MC, embT = s["phiT"], s["MC"], s["embT"]
                  e["psXT"] = []
                  for h in range(2):
                      pX = psB.tile([128, 2, 256], F32, tag="ps2", bufs=6,
                                    name=f"psXT_{b}_{c}_{h}")
                      pXv = pX[:].rearrange("p a b -> p (a b)")
                      nc.tensor.matmul(pXv, MC[:, 128 * h:128 * (h + 1)],
                                       phiT[:, c, :], start=True, stop=False)
                      nc.tensor.matmul(pXv, ct["ident"][:, :],
                                       embT[:, h, 512 * c:512 * (c + 1)],
                                       start=False, stop=True)
                      e["psXT"].append(pX)

              def ep_b(b, c):
                  """xT evac (Act)"""
                  s = st[b]
                  e = s["ep"][c]
                  xT = wp.tile([128, 2, 512], BF16, tag="xT", bufs=4,
                               name=f"xT_{b}_{c}")
                  e["xT"] = xT
                  nc.scalar.copy(xT[:, 0, :],
                                 e["psXT"][0][:].rearrange("p a b -> p (a b)"))
                  nc.vector.tensor_copy(
                      xT[:, 1, :], e["psXT"][1][:].rearrange("p a b -> p (a b)"))
                  e.pop("psXT")

              def ep_c(b, c):
                  """sq1 (DVE) + stats matmuls (PE)"""
                  s = st[b]
                  e = s["ep"][c]
                  xT = e["xT"]
                  sq = wp.tile([128, 2, 512], BF16, tag="sq", bufs=2,
                               name=f"sq_{b}_{c}")
                  nc.vector.tensor_mul(sq[:, 0, :], xT[:, 0, :], xT[:, 0, :])
                  nc.gpsimd.tensor_mul(sq[:, 1, :], xT[:, 1, :], xT[:, 1, :])
                  pS = psB.tile([128, 2, 256], F32, tag="ps2", bufs=6,
                                name=f"psS_{b}_{c}")
                  e["pS"] = pS
                  # one accumulation group at a time per 2KB zero region
                  for sub in range(4):
                      for h in range(2):
                          nc.tensor.matmul(
                              pS[:, 0, sub:sub + 1],
                              xT[:, h, 128 * sub:128 * (sub + 1)],
                              ct["onescol"][:, :],
                              start=(h == 0), stop=(h == 1))
                      for h in range(2):
                          nc.tensor.matmul(
                              pS[:, 0, 4 + sub:5 + sub],
                              sq[:, h, 128 * sub:128 * (sub + 1)],
                              ct["onescol"][:, :],
                              start=(h == 0), stop=(h == 1))
                      for h in range(2):
                          nc.tensor.matmul(
                              pS[:, 0, 8 + sub:9 + sub],
                              xT[:, h, 128 * sub:128 * (sub + 1)],
                              ct["wtil"][:, h:h + 1],
                              start=(h == 0), stop=(h == 1))

              def ep_d(b, c):
                  """LN1 stats math + mu row transpose (mu/var straight from
                  PSUM; e2 = eps*(var1+eps) folds r into the LN2 scale)"""
                  s = st[b]
                  e = s["ep"][c]
                  pS = e["pS"]
                  mu = tp.tile([128, 4], BF16, tag="mu", name=f"mu_{b}_{c}")
                  nc.scalar.activation(mu[:], pS[:, 0, 0:4], ACTF.Identity,
                                       scale=INVD)
                  var = tp.tile([128, 4], F32, tag="var", name=f"var_{b}_{c}")
                  nc.vector.tensor_mul(var[:], mu[:], mu[:])
                  nc.vector.scalar_tensor_tensor(
                      var[:], pS[:, 0, 4:8], INVD, var[:],
                      op0=ALU.mult, op1=ALU.subtract)
                  e2 = tp.tile([128, 4], F32, tag="e2", name=f"e2_{b}_{c}")
                  nc.gpsimd.tensor_scalar(e2[:], var[:], EPS, EPS * EPS,
                                          op0=ALU.mult, op1=ALU.add)
                  # analytic LN2 mean: negmu2 = -invD*(x.wtil) + mu*(sw*invD)
                  t1 = tp.tile([128, 4], F32, tag="t1", name=f"t1_{b}_{c}")
                  nc.gpsimd.tensor_scalar(t1[:], mu[:], ct["swd"][:, 0:1],
                                          None, op0=ALU.mult)
                  negmu2 = tp.tile([128, 4], F32, tag="negmu2",
                                   name=f"negmu2_{b}_{c}")
                  e["negmu2"] = negmu2
                  nc.vector.scalar_tensor_tensor(
                      negmu2[:], pS[:, 0, 8:12], -INVD, t1[:],
                      op0=ALU.mult, op1=ALU.add)
                  # m2e = negmu2^2 - e2  (so ep_h is svq*invD - m2e -> rsqrt)
                  m2e = tp.tile([128, 4], F32, tag="m2e", name=f"m2e_{b}_{c}")
                  e["m2e"] = m2e
                  nc.gpsimd.tensor_mul(m2e[:], negmu2[:], negmu2[:])
                  nc.gpsimd.tensor_sub(m2e[:], m2e[:], e2[:])
                  # transpose each mu column [128,1] -> [1,128] rows packed in
                  # the free dim (rank-1 lhsT needs base partition 0)
                  ptMu = psB.tile([128, 512], BF16, tag="psbf", bufs=2,
                                  name=f"ptMu_{b}_{c}")
                  for sub in range(4):
                      nc.tensor.transpose(ptMu[0:1, 128 * sub:128 * (sub + 1)],
                                          mu[:, sub:sub + 1],
                                          ct["ident"][:, :])
                  murow = tp.tile([1, 512], BF16, tag="murow",
                                  name=f"murow_{b}_{c}")
                  e["murow"] = murow
                  nc.vector.tensor_copy(murow[:], ptMu[0:1, 0:512])
                  e.pop("pS")

              def ep_e(b, c):
                  """psV = xT^T Wp - mu x wbar  (PE)"""
                  s = st[b]
                  e = s["ep"][c]
                  xT, murow = e["xT"], e["murow"]
                  e["psV"] = []
                  for p in range(2):
                      pV = psB.tile([128, 2, 256], F32, tag="ps2", bufs=6,
                                    name=f"psV_{b}_{c}_{p}")
                      e["psV"].append(pV)
                      for i in range(2):
                          sub = 2 * p + i
                          for h in range(2):
                              nc.tensor.matmul(
                                  pV[:, i, :],
                                  xT[:, h, 128 * sub:128 * (sub + 1)],
                                  ct["wo"][:, h, :],
                                  start=(h == 0), stop=False)
                          nc.tensor.matmul(pV[:, i, :],
                                           murow[0:1, 128 * sub:128 * (sub + 1)],
                                           ct["wbarneg"][0:1, :],
                                           start=False, stop=True)

              def ep_f(b, c):
                  """v evac + row sums (Act/DVE split)"""
                  s = st[b]
                  e = s["ep"][c]
                  v_bf = wp.tile([128, 4, 256], BF16, tag="v", bufs=4,
                                 name=f"v_{b}_{c}")
                  sv = tp.tile([128, 8], F32, tag="sv", name=f"sv_{b}_{c}")
                  e["v_bf"], e["sv"] = v_bf, sv
                  for sub in range(4):
                      pVs = e["psV"][sub // 2][:, sub % 2, :]
                      if sub % 2 == 0:
                          nc.scalar.copy(v_bf[:, sub, :], pVs)
                      else:
                          nc.vector.tensor_copy(v_bf[:, sub, :], pVs)
                  e.pop("psV")

              def ep_g(b, c):
                  """sumsq2 (Pool/DVE split)"""
                  s = st[b]
                  e = s["ep"][c]
                  v_bf, sv = e["v_bf"], e["sv"]
                  junk = wp.tile([128, 4, 256], BF16, tag="junk", bufs=2,
                                 name=f"junk_{b}_{c}")
                  for sub in range(4):
                      if sub < 2:
                          nc.scalar.activation(junk[:, sub, :], v_bf[:, sub, :],
                                               ACTF.Square,
                                               accum_out=sv[:, 4 + sub:5 + sub])
                      else:
                          nc.vector.scalar_tensor_tensor(
                              junk[:, sub, :], v_bf[:, sub, :], 1.0,
                              v_bf[:, sub, :], op0=ALU.mult, op1=ALU.mult,
                              accum_out=sv[:, 4 + sub:5 + sub])

              def ep_h(b, c):
                  """LN2 stats math: s = rsqrt(var2c + e2), e2 from ep_d"""
                  s = st[b]
                  e = s["ep"][c]
                  sv, m2e, negmu2 = e["sv"], e["m2e"], e["negmu2"]
                  var2 = tp.tile([128, 4], F32, tag="var2", name=f"var2_{b}_{c}")
                  nc.vector.scalar_tensor_tensor(
                      var2[:], sv[:, 4:8], INVD, m2e[:],
                      op0=ALU.mult, op1=ALU.subtract)
                  sfin = tp.tile([128, 4], F32, tag="sfin", name=f"sfin_{b}_{c}")
                  e["sfin"] = sfin
                  nc.scalar.activation(sfin[:], var2[:], ACTF.Sqrt)
                  nc.vector.reciprocal(sfin[:], sfin[:])
                  nms = tp.tile([128, 4], F32, tag="nms", name=f"nms_{b}_{c}")
                  e["nms"] = nms
                  nc.vector.tensor_mul(nms[:], negmu2[:], sfin[:])

              def ep_i(b, c):
                  """final normalize (DVE 4x TSP) + DMA out"""
                  s = st[b]
                  e = s["ep"][c]
                  v_bf, negmu2, sfin = e["v_bf"], e["negmu2"], e["sfin"]
                  nms = e["nms"]
                  ot = wp.tile([128, 4, 256], BF16, tag="ot", bufs=3,
                               name=f"ot_{b}_{c}")
                  for sub in range(4):
                      eng = nc.gpsimd if sub == 0 else nc.vector
                      eng.tensor_scalar(
                          ot[:, sub, :], v_bf[:, sub, :],
                          negmu2[:, sub:sub + 1], sfin[:, sub:sub + 1],
                          op0=ALU.add, op1=ALU.mult)
                      if flags["ln2_aff"]:
                          nc.vector.tensor_mul(ot[:, sub, :], ot[:, sub, :],
                                               ct["g2"][:, :])
                          nc.vector.tensor_add(ot[:, sub, :], ot[:, sub, :],
                                               ct["b2"][:, :])
                  nc.sync.dma_start(
                      out_d[b].rearrange("(t q) d -> q t d", q=128)
                           [:, 4 * c:4 * (c + 1), :], ot[:])
                  s["ep"].pop(c)

              # ---- emission: staggered batches ----
              # b0 stage1 -> [b1 stage1 || b0 diffusion] -> [b0 epilogue ||
              # b1 diffusion] -> b1 epilogue.  embT loads deferred so the
              # DMA FIFO serves pp8/emb first.
              for b in range(BL):
                  prologue(b)
              load_const("anch8")
              load_const("cb")
              load_emb(0)
              load_const("crow")
              load_const("cg")
              if flags["ln2_aff"]:
                  load_const("caff")
              load_emb(1)
              if "s1" in parts:
                  for b in range(BL):
                      stage1_init(b)

                  def s1_rotate(b, sph):
                      """pipeline the given stage-1 phases over the 8 chunks"""
                      for slot in range(8 + len(sph) - 1):
                          for k, ph in enumerate(sph):
                              u = slot - k
                              if 0 <= u < 8:
                                  ph(b, u)

                  s1_rotate(0, (s1a, s1b, s1c))
                  stage1_fin(0)
                  for b in range(BL):
                      load_embt(b)
                  # b1 phi chain (needs only pp8) first; then the emb-gated
                  # pC matmuls woven between b0 diffusion steps so neither
                  # stalls the in-order PE queue for long
                  do_diff = "diff" in parts

                  def s1_weave(b):
                      sph = (s1a, s1b, s1c)
                      for slot in range(8 + len(sph) - 1):
                          if do_diff and slot % 2 == 1 and slot // 2 < NUM_STEPS:
                              diffuse_step(0, slot // 2)
                          for k, ph in enumerate(sph):
                              u = slot - k
                              if 0 <= u < 8:
                                  ph(b, u)

                  s1_weave(1)
                  stage1_fin(1)
                  finish_coef(0)
                  if "epi" in parts:
                      units = [(0, c) for c in range(NC)] + \
                              [(1, c) for c in range(NC)]
                      nu = len(units)
                      phases = (ep_a, ep_b, ep_c, ep_d, ep_e, ep_f,
                                ep_g, ep_h, ep_i)
                      nst = len(phases)
                      # weave diffusion(b1) + finish_coef(1) into the early
                      # slots (b0 epilogue units), before b1 units need MC
                      extra2 = {}
                      if do_diff:
                          for sp in range(NUM_STEPS):
                              extra2[2 * sp] = [lambda s=sp: diffuse_step(1, s)]
                      extra2.setdefault(NUM_STEPS * 2 - 1, []).append(
                          lambda: finish_coef(1))
                      for slot in range(nu + nst - 1):
                          for fn in extra2.get(slot, ()):
                              fn()
                          for k, ph in enumerate(phases):
                              u = slot - k
                              if 0 <= u < nu:
                                  ph(*units[u])
                  else:
                      if do_diff:
                          for sp in range(NUM_STEPS):
                              diffuse_step(1, sp)
                      finish_coef(1)

    nc.compile()
    return nc


# --------------------------------------------------------------------------
# runner (same multi-core pjrt path as before)
# --------------------------------------------------------------------------
def _make_runner(nc):
    import jax
    import numpy as _np
    from jax.sharding import Mesh, PartitionSpec
    from jax.experimental.shard_map import shard_map
    from concourse import mybir as _mb
    from concourse.bass2jax import (install_neuronx_cc_hook, _bass_exec_p,
                                    partition_id_tensor)
    install_neuronx_cc_hook()
    partition_name = nc.partition_id_tensor.name if nc.partition_id_tensor else None
    in_names, out_names, out_avals, zero_outs = [], [], [], []
    for alloc in nc.m.functions[0].allocations:
        if not isinstance(alloc, _mb.MemoryLocationSet):
            continue
        name = alloc.memorylocations[0].name
        if alloc.kind == "ExternalInput":
            if name != partition_name:
                in_names.append(name)
        elif alloc.kind == "ExternalOutput":
            npdt = _mb.dt.np(alloc.dtype)
            out_names.append(name)
            out_avals.append(jax.core.ShapedArray(tuple(alloc.tensor_shape), npdt))
            zero_outs.append(_np.zeros(tuple(alloc.tensor_shape), npdt))
    n_params = len(in_names)
    n_outs = len(out_names)
    all_in = in_names + out_names + ([partition_name] if partition_name else [])

    def _body(*args):
        operands = list(args)
        if partition_name is not None:
            operands.append(partition_id_tensor())
        return tuple(_bass_exec_p.bind(
            *operands, out_avals=tuple(out_avals),
            in_names=tuple(all_in), out_names=tuple(out_names),
            lowering_input_output_aliases=(), sim_require_finite=True,
            sim_require_nnan=True, nc=nc))

    devices = jax.devices()[:NCORES]
    mesh = Mesh(_np.asarray(devices), ("core",))
    donate = tuple(range(n_params, n_params + n_outs))
    sharded = jax.jit(
        shard_map(_body, mesh=mesh,
                  in_specs=(PartitionSpec("core"),) * (n_params + n_outs),
                  out_specs=(PartitionSpec("core"),) * n_outs,
                  check_rep=False),
        donate_argnums=donate, keep_unused=True)

    def run(in_maps):
        per_core = [[_np.asarray(m[name]) for name in in_names] for m in in_maps]
        concat_in = [_np.concatenate([per_core[c][i] for c in range(NCORES)], axis=0)
                     for i in range(n_params)]
        concat_zero = [_np.zeros((NCORES * z.shape[0], *z.shape[1:]), z.dtype)
                       for z in zero_outs]
        outs = sharded(*concat_in, *concat_zero)
        outs = [_np.asarray(o) for o in outs]
        return {name: outs[i] for i, name in enumerate(out_names)}

    return run


def _host_pp8(pos):
    """pos [BL?, N] f32 -> pp8 [.., 8, N] fp16 rows [qh,qh,qlr,ph,ph,pl,1,1]"""
    p = np.asarray(pos, np.float32)
    ph = p.astype(np.float16).astype(np.float32)
    pl = ((p - ph) * 4096.0).astype(np.float16)
    qq = ph * ph
    qh = qq.astype(np.float16).astype(np.float32)
    qlr = ((qq - qh) * 2048.0 +
           ph * pl.astype(np.float32)).astype(np.float16)
    ones = np.ones_like(p, np.float16)
    return np.stack([qh.astype(np.float16), qh.astype(np.float16), qlr,
                     ph.astype(np.float16), ph.astype(np.float16), pl,
                     ones, ones], axis=-2)


def _host_embt(embb):
    """emb bf16 [B?, N, D] -> embT bf16 [B?, 2, 128, N]"""
    return np.ascontiguousarray(
        embb.transpose(0, 2, 1).reshape(embb.shape[0], 2, 128, N))


def kernel(**inputs):
    import ml_dtypes
    emb = np.ascontiguousarray(inputs["embeddings"], dtype=np.float32)
    pos = np.ascontiguousarray(inputs["positions"], dtype=np.float32)
    grid = np.asarray(inputs["grid_points"], np.float64)[0, :, 0]
    params = dict(
        sigma=float(np.asarray(inputs["sigma"])),
        alpha=float(np.asarray(inputs["alpha"])),
        grid=grid,
        W_int=np.asarray(inputs["W_int"], np.float64),
        b_int=np.asarray(inputs["b_int"], np.float64),
        W_out=np.asarray(inputs["W_out"], np.float64),
        b_out=np.asarray(inputs["b_out"], np.float64),
        ln1_g=np.asarray(inputs["ln1_g"], np.float64),
        ln1_b=np.asarray(inputs["ln1_b"], np.float64),
        ln2_g=np.asarray(inputs["ln2_g"], np.float64),
        ln2_b=np.asarray(inputs["ln2_b"], np.float64),
    )
    key = hashlib.sha256(b"".join(np.asarray(v).tobytes() for v in params.values())).hexdigest()
    if key not in _CACHE:
        consts, flags = _host_plan(**params)
        nc = _build_module(flags)
        _CACHE[key] = (_make_runner(nc), consts)
    run, consts = _CACHE[key]

    embb = emb.astype(ml_dtypes.bfloat16)
    embt = _host_embt(embb)
    pp8 = _host_pp8(pos[..., 0])                 # [B, 8, N]
    in_maps = []
    for c in range(NCORES):
        m = {"emb": embb[BL * c:BL * (c + 1)],
             "embT": embt[BL * c:BL * (c + 1)],
             "pp8": pp8[BL * c:BL * (c + 1)]}
        m.update(consts)
        in_maps.append(m)
    outs = run(in_maps)
    return np.ascontiguousarray(outs["out"].astype(np.float32))


# revision 61
# speedup vs baseline: 1.3130x; 1.0084x over previous
"""Trainium2 Bass kernel for nn_EnhancedTFNLayer (RBF field projection +
diffusion + sampling + LN/linear epilogue), data-parallel over batch on 8 cores.

Low-rank field pipeline (R=128 orthonormal basis Q fitted on host from the
parameter inputs only):

  phi[n, j] = exp(-(p_n - c_j)^2 / (2 s^2))   anchor features (fp16
              split-precision K=8 matmul + Exp)
  C = Wq^T (phi^T emb)                        field coords
  4x diffusion: C' = SLQ C + QTW @ tanh(Qsub^T (C W_int) + b_int)
  sampledT = (MQ C)^T phi  computed D-major: psXT = MC_h^T phiT + I^T embT
  LN1 folded into the W matmul:  v_raw = x @ Wp - mu_t * wbar   (rank-1 PE
  matmul with mu transposed to a row); r_t folds into the final LN2 scale
  s_t = r * rsqrt(r^2 * var2c + eps).  LN1 stats come from 1-column PE
  matmuls (stationary xT / xT^2 pieces x ones column) in token-major form.
"""
import sys
import hashlib
import numpy as np

for _p in ("/opt/trn_rl_repo", "/root/.axon_site/_ro/trn_rl_repo"):
    if _p not in sys.path:
        sys.path.insert(0, _p)

import concourse.bass as bass
import concourse.bacc as bacc
import concourse.tile as tile
from concourse import mybir

F32 = mybir.dt.float32
BF16 = mybir.dt.bfloat16
FP16 = mybir.dt.float16
ACTF = mybir.ActivationFunctionType
ALU = mybir.AluOpType
AXL = mybir.AxisListType

B, N, G, D = 16, 4096, 1024, 256
NUM_STEPS, DT, EPS = 4, 0.01, 1e-5
R = 128
SSUB = 256               # tanh-subsampled grid points
NT = N // 128            # 32 token tiles per batch
NC = N // 512            # 8 chunks of 512 tokens per batch
BL = 2                   # batches per core
NCORES = 8
INVD = 1.0 / D

_CACHE = {}


def _fp16(x):
    return np.float16(np.asarray(x, np.float64).astype(np.float32)).astype(np.float32)


# --------------------------------------------------------------------------
# host-side operator fitting (float64; parameter inputs only)
# --------------------------------------------------------------------------
def _host_plan(sigma, alpha, grid, W_int, b_int, W_out, b_out,
               ln1_g, ln1_b, ln2_g, ln2_b):
    rng = np.random.default_rng(0)
    c0 = 1.0 - 2.0 * alpha * DT
    c1 = alpha * DT
    pg = np.linspace(0.0, 1.0, 8193)
    K = np.exp(-((pg[:, None] - grid[None, :]) ** 2) / (2 * sigma * sigma))
    nsyn = 384
    sub = rng.choice(len(pg), size=256, replace=False)
    Fsyn = K[sub].T @ rng.standard_normal((256, nsyn))
    Fsyn /= np.abs(Fsyn).max(0, keepdims=True) + 1e-30
    fscale = np.sqrt(N * sigma * np.sqrt(np.pi))
    wnorm = np.linalg.norm(W_int, axis=0)
    wcols = rng.choice(len(wnorm), size=nsyn)
    gains = fscale * wnorm[wcols] * rng.uniform(0.5, 2.0, nsyn)
    Tsyn = np.tanh(Fsyn * gains[None, :])
    Msvd = np.concatenate([K, (Tsyn * 0.1).T], axis=0)
    _, _, Vt = np.linalg.svd(Msvd, full_matrices=False)
    Q = Vt[:R]                                            # [R, G] orthonormal
    # anchors
    c = np.linspace(-0.08, 1.08, R)
    s = 2.2 * (c[1] - c[0])
    F = np.exp(-((pg[:, None] - c[None, :]) ** 2) / (2 * s * s))
    Qk = K @ Q.T
    Wq, *_ = np.linalg.lstsq(F, Qk, rcond=1e-8)           # [R, R]
    Qt = Q.T
    LQt = c0 * Qt.copy()
    LQt[1:-1] += c1 * (Qt[:-2] + Qt[2:])
    LQt[0] += c1 * (Qt[0] + Qt[1])
    LQt[-1] += c1 * (Qt[-2] + Qt[-1])
    SLQ = Q @ LQt                                         # [R, R]
    u = pg * (G - 1)
    i0 = np.clip(np.floor(u), 0, G - 2).astype(int)
    w = u - i0
    lerpQ = Qt[i0] * (1 - w)[:, None] + Qt[i0 + 1] * w[:, None]
    MQ, *_ = np.linalg.lstsq(F, lerpQ, rcond=1e-5)        # [R, R]

    # subsampled-tanh quadrature back-projection QTW [R, SSUB]
    subidx = np.unique(np.linspace(0, G - 1, SSUB).round().astype(int))
    assert len(subidx) == SSUB
    nsyn2 = 1024
    Fg = np.exp(-((grid[:, None] - grid[None, ::8]) ** 2) / (2 * sigma * sigma))
    fields = Fg @ rng.standard_normal((Fg.shape[1], nsyn2))
    fields /= np.abs(fields).max(0, keepdims=True) + 1e-30
    gains2 = fscale * wnorm[rng.choice(len(wnorm), size=nsyn2)] * \
        np.exp(rng.uniform(np.log(0.25), np.log(4.0), nsyn2))
    TG = np.tanh(fields * gains2[None, :])                # [G, nsyn2]
    target = Q @ TG
    A = TG[subidx, :]
    lam = 1e-6 * np.linalg.norm(A) ** 2 / A.shape[0]
    QTW = np.linalg.solve(A @ A.T + lam * np.eye(SSUB), A @ target.T).T

    # fp16 split-precision anchor coefficient matrix [8, R]
    # pp8 rows on device: [qh, qh, qlr, ph, ph, pl, 1, 1]
    a3 = -1.0 / (2 * s * s)
    a1 = c / (s * s)
    a2 = -c * c / (2 * s * s)
    a3h = _fp16(a3); a3l = a3 - a3h
    a1h = _fp16(a1); a1l = a1 - a1h
    a2h = _fp16(a2); a2l = a2 - a2h
    anch8 = np.stack([
        np.full(R, a3h), np.full(R, a3l), np.full(R, a3 / 2048.0),
        a1h, a1l, a1 / 4096.0,
        a2h, a2l,
    ], axis=0)

    # affine folds: enh_aff = enh*g1 + b1 ; v = enh_aff @ (W_out + I) + b_out
    Wp = ln1_g[:, None] * (W_out + np.eye(D))             # rows scaled by g1
    brow = b_out + ln1_b @ (W_out + np.eye(D))            # const row
    wbar = Wp.sum(axis=0)                                 # column sums [D]
    f32 = lambda x: np.ascontiguousarray(x, dtype=np.float32)
    f16 = lambda x: np.ascontiguousarray(x, dtype=np.float16)

    qsub = Q[:, subidx]                                   # [R, SSUB]
    qtw_t = (QTW * DT).T.reshape(2, 128, R).transpose(1, 0, 2)  # [128,2,R]
    wi = W_int.reshape(2, 128, D).transpose(1, 0, 2)      # [128,2,D]
    wo = Wp.reshape(2, 128, D).transpose(1, 0, 2)         # [128,2,D]
    onescol = np.ones((128, 1))
    wtil = Wp.sum(axis=1).reshape(2, 128).T              # [128, 2] row sums
    cb = np.concatenate([
        qsub,                                             # [:,0:256]
        qtw_t.reshape(128, 2 * R),                        # [:,256:512]
        SLQ.T, Wq, MQ.T,                                  # 512:640,640:768,768:896
        wi.reshape(128, 2 * D),                           # 896:1408
        wo.reshape(128, 2 * D),                           # 1408:1920
        np.eye(128),                                      # 1920:2048
        onescol,                                          # 2048:2049
        wtil,                                             # 2049:2051
    ], axis=1)
    # row blob (bf16) [1, 896]: bint row | brow | ones128 | -wbar
    crow = np.concatenate([
        b_int.reshape(1, D), brow.reshape(1, D), np.ones((1, 128)),
        -wbar.reshape(1, D),
    ], axis=1)
    swD = float(wbar.sum()) * (1.0 / D)
    cg = np.concatenate([np.full((128, 1), EPS), np.eye(128),
                         np.full((128, 1), swD)], axis=1)
    caff = np.concatenate([np.broadcast_to(ln2_g, (128, D)),
                           np.broadcast_to(ln2_b, (128, D))], axis=1)

    import ml_dtypes
    bfl = lambda x: np.ascontiguousarray(x, dtype=ml_dtypes.bfloat16)
    consts = {
        "anch8": f16(anch8),
        "cb": bfl(cb),
        "crow": bfl(crow),
        "cg": f32(cg),
        "caff": f32(caff),
    }
    flags = {
        "use_bint": bool(np.any(b_int != 0)),
        "use_brow": bool(np.any(np.abs(brow) > 1e-12)),
        "ln2_aff": bool(np.any(ln2_g != 1) or np.any(ln2_b != 0)),
    }
    return consts, flags


# --------------------------------------------------------------------------
# device module
# --------------------------------------------------------------------------
def _build_module(flags, repeats=1, parts=("s1", "diff", "epi")):
    nc = bacc.Bacc(trn_type="TRN2")
    emb_d = nc.dram_tensor("emb", [BL, N, D], BF16, kind="ExternalInput")
    embt_d = nc.dram_tensor("embT", [BL, 2, 128, N], BF16, kind="ExternalInput")
    pp8_d = nc.dram_tensor("pp8", [BL, 8, N], FP16, kind="ExternalInput")
    const_specs = {
        "anch8": ([8, R], FP16),
        "cb": ([128, 2051], BF16),
        "crow": ([1, 3 * D + 128], BF16),
        "cg": ([128, 130], F32),
        "caff": ([128, 2 * D], F32),
    }
    cd = {k: nc.dram_tensor(k, sh, dt, kind="ExternalInput")
          for k, (sh, dt) in const_specs.items()}
    out_d = nc.dram_tensor("out", [BL, N, D], BF16, kind="ExternalOutput")

    with tile.TileContext(nc) as tc:
        with tc.tile_pool(name="consts", bufs=1) as cp, \
             tc.tile_pool(name="emb", bufs=2) as embp, \
             tc.tile_pool(name="phi", bufs=2) as phip, \
             tc.tile_pool(name="coef", bufs=2) as coefp, \
             tc.tile_pool(name="pre", bufs=2) as prep, \
             tc.tile_pool(name="work", bufs=3) as wp, \
             tc.tile_pool(name="tiny", bufs=8) as tp, \
             tc.tile_pool(name="psB", bufs=1, space="PSUM") as psB:

            # ---- constants (tiles allocated here; DMAs emitted in the
            # priority order interleaved with input loads below) ----
            blob = {}
            for k, (sh, dt) in const_specs.items():
                if k == "caff" and not flags["ln2_aff"]:
                    continue
                blob[k] = cp.tile(sh, dt, tag=k, name=f"c_{k}")

            def load_const(k):
                sh = const_specs[k][0]
                nc.sync.dma_start(blob[k][:], cd[k][tuple(slice(None) for _ in sh)])

            _cb = blob["cb"]
            ct = {
                "anch8": blob["anch8"],
                "qsub": _cb[:, 0:256],
                "qtw": _cb[:, 256:512].rearrange("p (a b) -> p a b", a=2),
                "slt": _cb[:, 512:640], "wq": _cb[:, 640:768],
                "mqt": _cb[:, 768:896],
                "wi": _cb[:, 896:1408].rearrange("p (a b) -> p a b", a=2),
                "wo": _cb[:, 1408:1920].rearrange("p (a b) -> p a b", a=2),
                "ident": _cb[:, 1920:2048],
                "onescol": _cb[:, 2048:2049],
                "wtil": _cb[:, 2049:2051],
                "bint_row": blob["crow"][:, 0:D],
                "brow": blob["crow"][:, D:2 * D],
                "ones1": blob["crow"][:, 2 * D:2 * D + 128],
                "wbarneg": blob["crow"][:, 2 * D + 128:3 * D + 128],
                "epsb": blob["cg"][:, 0:1],
                "identf": blob["cg"][:, 1:129],
                "swd": blob["cg"][:, 129:130],
            }
            if flags["ln2_aff"]:
                ct["g2"] = blob["caff"][:, 0:D]
                ct["b2"] = blob["caff"][:, D:2 * D]

            import contextlib
            loopctx = tc.For_i(0, repeats, 1) if repeats > 1 else contextlib.nullcontext()
            with loopctx:
              st = [dict() for _ in range(BL)]

              def load_emb(b):
                  s = st[b]
                  s["emb"] = embp.tile([128, NT, D], BF16, tag="emb",
                                       name=f"emb_{b}")
                  eap = emb_d[b].rearrange("(t q) d -> q t d", q=128)
                  for k4 in range(4):
                      nc.sync.dma_start(s["emb"][:, 8 * k4:8 * (k4 + 1), :],
                                        eap[:, 8 * k4:8 * (k4 + 1), :])

              def load_embt(b):
                  # same SP queue, emitted after the emb loads so the shared
                  # DMA FIFO serves stage1's inputs first
                  s = st[b]
                  s["embT"] = embp.tile([128, 2, N], BF16, tag="embT",
                                        name=f"embT_{b}")
                  etap = embt_d[b].rearrange("h q t -> q h t")
                  for h in range(2):
                      nc.sync.dma_start(s["embT"][:, h, :], etap[:, h, :])

              def prologue(b):
                  """pp8 rows [qh, qh, qlr, ph, ph, pl, 1, 1] host-computed."""
                  s = st[b]
                  pp8 = prep.tile([8, N], FP16, tag="pp8", name=f"pp8_{b}")
                  nc.sync.dma_start(pp8[:], pp8_d[b])
                  s["pp8"] = pp8

              def stage1_init(b):
                  s = st[b]
                  phiT = phip.tile([R, 8, 512], BF16, tag="phiT", name=f"phiT_{b}")
                  phiN = phip.tile([128, NT, 128], BF16, tag="phiN",
                                   name=f"phiN_{b}")
                  s["phiT"], s["phiN"] = phiT, phiN
                  s["pCt"] = psB.tile([128, 2, 256], F32, tag="ps2", bufs=6,
                                      name=f"pC_{b}")

              def s1a(b, j):
                  """psPhi matmul + Exp"""
                  s = st[b]
                  pp8, phiT = s["pp8"], s["phiT"]
                  psPhi = psB.tile([128, 2, 256], F32, tag="ps2", bufs=6,
                                   name=f"psPhi_{b}_{j}")
                  psPhiv = psPhi[:].rearrange("p a b -> p (a b)")
                  nc.tensor.matmul(psPhiv, ct["anch8"][:, :],
                                   pp8[:, 512 * j:512 * (j + 1)],
                                   start=True, stop=True)
                  nc.scalar.activation(phiT[:, j, :], psPhiv, ACTF.Exp)

              def s1b(b, j):
                  """phiT transposes + evac to phiN"""
                  s = st[b]
                  phiT, phiN = s["phiT"], s["phiN"]
                  ptT = psB.tile([128, 512], BF16, tag="psbf", bufs=2,
                                 name=f"ptT_{b}_{j}")
                  for h in range(4):
                      nc.tensor.transpose(ptT[:, 128 * h:128 * (h + 1)],
                                          phiT[:, j, 128 * h:128 * (h + 1)],
                                          ct["ident"][:, :])
                  # evac on DVE (2x mode on bf16) -- Act is busy with Exp here
                  dst = phiN[:, 4 * j:4 * (j + 1), :].rearrange("p a b -> p (a b)")
                  nc.vector.tensor_copy(dst, ptT[:])

              def s1c(b, j):
                  """pC accumulation matmuls"""
                  s = st[b]
                  emb_sb, phiN = s["emb"], s["phiN"]
                  pC = s["pCt"][:, 0, :]
                  for h in range(4):
                      t = 4 * j + h
                      nc.tensor.matmul(pC, phiN[:, t, :], emb_sb[:, t, :],
                                       start=(t == 0), stop=(t == NT - 1))

              def stage1_fin(b):
                  s = st[b]
                  pC = s["pCt"][:, 0, :]
                  craw = coefp.tile([R, D], BF16, tag="craw", name=f"craw_{b}")
                  nc.scalar.copy(craw[:], pC)
                  pC2t = psB.tile([128, 2, 256], F32, tag="ps2", bufs=6,
                                  name=f"pC2_{b}")
                  pC2 = pC2t[:, 0, :]
                  nc.tensor.matmul(pC2, ct["wq"][:, :], craw[:],
                                   start=True, stop=True)
                  C = coefp.tile([R, D], BF16, tag="C", bufs=4, name=f"C_{b}")
                  nc.vector.tensor_copy(C[:], pC2)
                  s["C"] = C

              def diffuse_step(b, step):
                  s = st[b]
                  C = s["C"]
                  ptC = psB.tile([128, 512], BF16, tag="psbf", bufs=2,
                                 name=f"ptC_{b}_{step}")
                  for h in range(2):
                      nc.tensor.transpose(ptC[:, 128 * h:128 * (h + 1)],
                                          C[:, 128 * h:128 * (h + 1)],
                                          ct["ident"][:, :])
                  Ct = wp.tile([128, 2, 128], BF16, tag="Ct", name=f"Ct_{b}_{step}")
                  nc.vector.tensor_copy(
                      Ct[:].rearrange("p a b -> p (a b)"), ptC[:, 0:256])
                  pCWt = psB.tile([128, 2, 256], F32, tag="ps2", bufs=6,
                                  name=f"pCW_{b}_{step}")
                  pCW = pCWt[:, 0, :]
                  for h in range(2):
                      nc.tensor.matmul(pCW, Ct[:, h, :], ct["wi"][:, h, :],
                                       start=(h == 0), stop=(h == 1))
                  CWb = wp.tile([R, D], BF16, tag="CWb", name=f"CWb_{b}_{step}")
                  nc.scalar.copy(CWb[:], pCW)
                  psF = psB.tile([128, 2, 256], F32, tag="ps2", bufs=6,
                                 name=f"psF_{b}_{step}")
                  for sc in range(2):
                      nc.tensor.matmul(psF[:, sc, :],
                                       ct["qsub"][:, 128 * sc:128 * (sc + 1)],
                                       CWb[:], start=True,
                                       stop=not flags["use_bint"])
                      if flags["use_bint"]:
                          nc.tensor.matmul(psF[:, sc, :], ct["ones1"][0:1, :],
                                           ct["bint_row"][0:1, :],
                                           start=False, stop=True)
                  T = wp.tile([128, 2, 256], BF16, tag="T", name=f"T_{b}_{step}")
                  nc.scalar.activation(T[:].rearrange("p a b -> p (a b)"),
                                       psF[:].rearrange("p a b -> p (a b)"),
                                       ACTF.Tanh)
                  pCnt = psB.tile([128, 2, 256], F32, tag="ps2", bufs=6,
                                  name=f"pCn_{b}_{step}")
                  pCn = pCnt[:, 0, :]
                  nc.tensor.matmul(pCn, ct["slt"][:, :], C[:],
                                   start=True, stop=False)
                  for sc in range(2):
                      nc.tensor.matmul(pCn, ct["qtw"][:, sc, :], T[:, sc, :],
                                       start=False, stop=(sc == 1))
                  C2 = coefp.tile([R, D], BF16, tag="C", bufs=4,
                                  name=f"C_{b}_{step}")
                  nc.vector.tensor_copy(C2[:], pCn)
                  s["C"] = C2

              def finish_coef(b):
                  s = st[b]
                  pMCt = psB.tile([128, 2, 256], F32, tag="ps2", bufs=6,
                                  name=f"pMC_{b}")
                  pMC = pMCt[:, 0, :]
                  nc.tensor.matmul(pMC, ct["mqt"][:, :], s["C"][:],
                                   start=True, stop=True)
                  MC = coefp.tile([R, D], BF16, tag="MC", name=f"MC_{b}")
                  nc.scalar.copy(MC[:], pMC)
                  s["MC"] = MC

              # ---- epilogue v2: D-major sampled, LN1 folded into W matmul ---
              def ep_a(b, c):
                  """psXT_h = MC_h^T phiT_chunk + I^T embT_h  (PE)"""
                  s = st[b]
                  e = s.setdefault("ep", {}).setdefault(c, {})
                  phiT, MC, embT = s["phiT"], s["MC"], s["embT"]
                  e["psXT"] = []
                  for h in range(2):
                      pX = psB.tile([128, 2, 256], F32, tag="ps2", bufs=6,
                                    name=f"psXT_{b}_{c}_{h}")
                      pXv = pX[:].rearrange("p a b -> p (a b)")
                      nc.tensor.matmul(pXv, MC[:, 128 * h:128 * (h + 1)],
                                       phiT[:, c, :], start=True, stop=False)
                      nc.tensor.matmul(pXv, ct["ident"][:, :],
                                       embT[:, h, 512 * c:512 * (c + 1)],
                                       start=False, stop=True)
                      e["psXT"].append(pX)

              def ep_b(b, c):
                  """xT evac (Act)"""
                  s = st[b]
                  e = s["ep"][c]
                  xT = wp.tile([128, 2, 512], BF16, tag="xT", bufs=4,
                               name=f"xT_{b}_{c}")
                  e["xT"] = xT
                  nc.scalar.copy(xT[:, 0, :],
                                 e["psXT"][0][:].rearrange("p a b -> p (a b)"))
                  nc.vector.tensor_copy(
                      xT[:, 1, :], e["psXT"][1][:].rearrange("p a b -> p (a b)"))
                  e.pop("psXT")

              def ep_c(b, c):
                  """sq1 (DVE) + stats matmuls (PE)"""
                  s = st[b]
                  e = s["ep"][c]
                  xT = e["xT"]
                  sq = wp.tile([128, 2, 512], BF16, tag="sq", bufs=3,
                               name=f"sq_{b}_{c}")
                  nc.vector.tensor_mul(sq[:, 0, :], xT[:, 0, :], xT[:, 0, :])
                  nc.gpsimd.tensor_mul(sq[:, 1, :], xT[:, 1, :], xT[:, 1, :])
                  pS = psB.tile([128, 2, 256], F32, tag="ps2", bufs=6,
                                name=f"psS_{b}_{c}")
                  e["pS"] = pS
                  # one accumulation group at a time per 2KB zero region
                  for sub in range(4):
                      for h in range(2):
                          nc.tensor.matmul(
                              pS[:, 0, sub:sub + 1],
                              xT[:, h, 128 * sub:128 * (sub + 1)],
                              ct["onescol"][:, :],
                              start=(h == 0), stop=(h == 1))
                      for h in range(2):
                          nc.tensor.matmul(
                              pS[:, 0, 4 + sub:5 + sub],
                              sq[:, h, 128 * sub:128 * (sub + 1)],
                              ct["onescol"][:, :],
                              start=(h == 0), stop=(h == 1))
                      for h in range(2):
                          nc.tensor.matmul(
                              pS[:, 0, 8 + sub:9 + sub],
                              xT[:, h, 128 * sub:128 * (sub + 1)],
                              ct["wtil"][:, h:h + 1],
                              start=(h == 0), stop=(h == 1))

              def ep_d(b, c):
                  """LN1 stats math + mu row transpose (mu/var straight from
                  PSUM; e2 = eps*(var1+eps) folds r into the LN2 scale)"""
                  s = st[b]
                  e = s["ep"][c]
                  pS = e["pS"]
                  mu = tp.tile([128, 4], BF16, tag="mu", name=f"mu_{b}_{c}")
                  nc.scalar.activation(mu[:], pS[:, 0, 0:4], ACTF.Identity,
                                       scale=INVD)
                  var = tp.tile([128, 4], F32, tag="var", name=f"var_{b}_{c}")
                  nc.vector.tensor_mul(var[:], mu[:], mu[:])
                  nc.vector.scalar_tensor_tensor(
                      var[:], pS[:, 0, 4:8], INVD, var[:],
                      op0=ALU.mult, op1=ALU.subtract)
                  e2 = tp.tile([128, 4], F32, tag="e2", name=f"e2_{b}_{c}")
                  nc.gpsimd.tensor_scalar(e2[:], var[:], EPS, EPS * EPS,
                                          op0=ALU.mult, op1=ALU.add)
                  # analytic LN2 mean: negmu2 = -invD*(x.wtil) + mu*(sw*invD)
                  t1 = tp.tile([128, 4], F32, tag="t1", name=f"t1_{b}_{c}")
                  nc.gpsimd.tensor_scalar(t1[:], mu[:], ct["swd"][:, 0:1],
                                          None, op0=ALU.mult)
                  negmu2 = tp.tile([128, 4], F32, tag="negmu2",
                                   name=f"negmu2_{b}_{c}")
                  e["negmu2"] = negmu2
                  nc.vector.scalar_tensor_tensor(
                      negmu2[:], pS[:, 0, 8:12], -INVD, t1[:],
                      op0=ALU.mult, op1=ALU.add)
                  # m2e = negmu2^2 - e2  (so ep_h is svq*invD - m2e -> rsqrt)
                  m2e = tp.tile([128, 4], F32, tag="m2e", name=f"m2e_{b}_{c}")
                  e["m2e"] = m2e
                  nc.gpsimd.tensor_mul(m2e[:], negmu2[:], negmu2[:])
                  nc.gpsimd.tensor_sub(m2e[:], m2e[:], e2[:])
                  # transpose each mu column [128,1] -> [1,128] rows packed in
                  # the free dim (rank-1 lhsT needs base partition 0)
                  ptMu = psB.tile([128, 512], BF16, tag="psbf", bufs=2,
                                  name=f"ptMu_{b}_{c}")
                  for sub in range(4):
                      nc.tensor.transpose(ptMu[0:1, 128 * sub:128 * (sub + 1)],
                                          mu[:, sub:sub + 1],
                                          ct["ident"][:, :])
                  murow = tp.tile([1, 512], BF16, tag="murow",
                                  name=f"murow_{b}_{c}")
                  e["murow"] = murow
                  nc.vector.tensor_copy(murow[:], ptMu[0:1, 0:512])
                  e.pop("pS")

              def ep_e(b, c):
                  """psV = xT^T Wp - mu x wbar  (PE)"""
                  s = st[b]
                  e = s["ep"][c]
                  xT, murow = e["xT"], e["murow"]
                  e["psV"] = []
                  for p in range(2):
                      pV = psB.tile([128, 2, 256], F32, tag="ps2", bufs=6,
                                    name=f"psV_{b}_{c}_{p}")
                      e["psV"].append(pV)
                      for i in range(2):
                          sub = 2 * p + i
                          for h in range(2):
                              nc.tensor.matmul(
                                  pV[:, i, :],
                                  xT[:, h, 128 * sub:128 * (sub + 1)],
                                  ct["wo"][:, h, :],
                                  start=(h == 0), stop=False)
                          nc.tensor.matmul(pV[:, i, :],
                                           murow[0:1, 128 * sub:128 * (sub + 1)],
                                           ct["wbarneg"][0:1, :],
                                           start=False, stop=True)

              def ep_f(b, c):
                  """v evac + row sums (Act/DVE split)"""
                  s = st[b]
                  e = s["ep"][c]
                  v_bf = wp.tile([128, 4, 256], BF16, tag="v", bufs=4,
                                 name=f"v_{b}_{c}")
                  sv = tp.tile([128, 8], F32, tag="sv", name=f"sv_{b}_{c}")
                  e["v_bf"], e["sv"] = v_bf, sv
                  for sub in range(4):
                      pVs = e["psV"][sub // 2][:, sub % 2, :]
                      if sub % 2 == 0:
                          nc.scalar.copy(v_bf[:, sub, :], pVs)
                      else:
                          nc.vector.tensor_copy(v_bf[:, sub, :], pVs)
                  e.pop("psV")

              def ep_g(b, c):
                  """sumsq2 (Pool/DVE split)"""
                  s = st[b]
                  e = s["ep"][c]
                  v_bf, sv = e["v_bf"], e["sv"]
                  junk = wp.tile([128, 4, 256], BF16, tag="junk", bufs=3,
                                 name=f"junk_{b}_{c}")
                  for sub in range(4):
                      if sub < 2:
                          nc.scalar.activation(junk[:, sub, :], v_bf[:, sub, :],
                                               ACTF.Square,
                                               accum_out=sv[:, 4 + sub:5 + sub])
                      else:
                          nc.vector.scalar_tensor_tensor(
                              junk[:, sub, :], v_bf[:, sub, :], 1.0,
                              v_bf[:, sub, :], op0=ALU.mult, op1=ALU.mult,
                              accum_out=sv[:, 4 + sub:5 + sub])

              def ep_h(b, c):
                  """LN2 stats math: s = rsqrt(var2c + e2), e2 from ep_d"""
                  s = st[b]
                  e = s["ep"][c]
                  sv, m2e, negmu2 = e["sv"], e["m2e"], e["negmu2"]
                  var2 = tp.tile([128, 4], F32, tag="var2", name=f"var2_{b}_{c}")
                  nc.vector.scalar_tensor_tensor(
                      var2[:], sv[:, 4:8], INVD, m2e[:],
                      op0=ALU.mult, op1=ALU.subtract)
                  sfin = tp.tile([128, 4], F32, tag="sfin", name=f"sfin_{b}_{c}")
                  e["sfin"] = sfin
                  nc.scalar.activation(sfin[:], var2[:], ACTF.Sqrt)
                  nc.vector.reciprocal(sfin[:], sfin[:])
                  nms = tp.tile([128, 4], F32, tag="nms", name=f"nms_{b}_{c}")
                  e["nms"] = nms
                  nc.vector.tensor_mul(nms[:], negmu2[:], sfin[:])

              def ep_i(b, c):
                  """final normalize (DVE 4x TSP) + DMA out"""
                  s = st[b]
                  e = s["ep"][c]
                  v_bf, negmu2, sfin = e["v_bf"], e["negmu2"], e["sfin"]
                  nms = e["nms"]
                  ot = wp.tile([128, 4, 256], BF16, tag="ot", bufs=4,
                               name=f"ot_{b}_{c}")
                  for sub in range(4):
                      eng = nc.gpsimd if sub == 0 else nc.vector
                      eng.tensor_scalar(
                          ot[:, sub, :], v_bf[:, sub, :],
                          negmu2[:, sub:sub + 1], sfin[:, sub:sub + 1],
                          op0=ALU.add, op1=ALU.mult)
                      if flags["ln2_aff"]:
                          nc.vector.tensor_mul(ot[:, sub, :], ot[:, sub, :],
                                               ct["g2"][:, :])
                          nc.vector.tensor_add(ot[:, sub, :], ot[:, sub, :],
                                               ct["b2"][:, :])
                  nc.sync.dma_start(
                      out_d[b].rearrange("(t q) d -> q t d", q=128)
                           [:, 4 * c:4 * (c + 1), :], ot[:])
                  s["ep"].pop(c)

              # ---- emission: staggered batches ----
              # b0 stage1 -> [b1 stage1 || b0 diffusion] -> [b0 epilogue ||
              # b1 diffusion] -> b1 epilogue.  embT loads deferred so the
              # DMA FIFO serves pp8/emb first.
              for b in range(BL):
                  prologue(b)
              load_const("anch8")
              load_const("cb")
              load_emb(0)
              load_const("crow")
              load_const("cg")
              if flags["ln2_aff"]:
                  load_const("caff")
              load_emb(1)
              if "s1" in parts:
                  for b in range(BL):
                      stage1_init(b)

                  def s1_rotate(b, sph):
                      """pipeline the given stage-1 phases over the 8 chunks"""
                      for slot in range(8 + len(sph) - 1):
                          for k, ph in enumerate(sph):
                              u = slot - k
                              if 0 <= u < 8:
                                  ph(b, u)

                  s1_rotate(0, (s1a, s1b, s1c))
                  stage1_fin(0)
                  for b in range(BL):
                      load_embt(b)
                  # b1 phi chain (needs only pp8) first; then the emb-gated
                  # pC matmuls woven between b0 diffusion steps so neither
                  # stalls the in-order PE queue for long
                  do_diff = "diff" in parts

                  def s1_weave(b):
                      sph = (s1a, s1b, s1c)
                      for slot in range(8 + len(sph) - 1):
                          if do_diff and slot % 2 == 1 and slot // 2 < NUM_STEPS:
                              diffuse_step(0, slot // 2)
                          for k, ph in enumerate(sph):
                              u = slot - k
                              if 0 <= u < 8:
                                  ph(b, u)

                  s1_weave(1)
                  stage1_fin(1)
                  finish_coef(0)
                  if "epi" in parts:
                      units = [(0, c) for c in range(NC)] + \
                              [(1, c) for c in range(NC)]
                      nu = len(units)
                      phases = (ep_a, ep_b, ep_c, ep_d, ep_e, ep_f,
                                ep_g, ep_h, ep_i)
                      nst = len(phases)
                      # weave diffusion(b1) + finish_coef(1) into the early
                      # slots (b0 epilogue units), before b1 units need MC
                      extra2 = {}
                      if do_diff:
                          for sp in range(NUM_STEPS):
                              extra2[2 * sp] = [lambda s=sp: diffuse_step(1, s)]
                      extra2.setdefault(NUM_STEPS * 2 - 1, []).append(
                          lambda: finish_coef(1))
                      for slot in range(nu + nst - 1):
                          for fn in extra2.get(slot, ()):
                              fn()
                          for k, ph in enumerate(phases):
                              u = slot - k
                              if 0 <= u < nu:
                                  ph(*units[u])
                  else:
                      if do_diff:
                          for sp in range(NUM_STEPS):
                              diffuse_step(1, sp)
                      finish_coef(1)

    nc.compile()
    return nc


# --------------------------------------------------------------------------
# runner (same multi-core pjrt path as before)
# --------------------------------------------------------------------------
def _make_runner(nc):
    import jax
    import numpy as _np
    from jax.sharding import Mesh, PartitionSpec
    from jax.experimental.shard_map import shard_map
    from concourse import mybir as _mb
    from concourse.bass2jax import (install_neuronx_cc_hook, _bass_exec_p,
                                    partition_id_tensor)
    install_neuronx_cc_hook()
    partition_name = nc.partition_id_tensor.name if nc.partition_id_tensor else None
    in_names, out_names, out_avals, zero_outs = [], [], [], []
    for alloc in nc.m.functions[0].allocations:
        if not isinstance(alloc, _mb.MemoryLocationSet):
            continue
        name = alloc.memorylocations[0].name
        if alloc.kind == "ExternalInput":
            if name != partition_name:
                in_names.append(name)
        elif alloc.kind == "ExternalOutput":
            npdt = _mb.dt.np(alloc.dtype)
            out_names.append(name)
            out_avals.append(jax.core.ShapedArray(tuple(alloc.tensor_shape), npdt))
            zero_outs.append(_np.zeros(tuple(alloc.tensor_shape), npdt))
    n_params = len(in_names)
    n_outs = len(out_names)
    all_in = in_names + out_names + ([partition_name] if partition_name else [])

    def _body(*args):
        operands = list(args)
        if partition_name is not None:
            operands.append(partition_id_tensor())
        return tuple(_bass_exec_p.bind(
            *operands, out_avals=tuple(out_avals),
            in_names=tuple(all_in), out_names=tuple(out_names),
            lowering_input_output_aliases=(), sim_require_finite=True,
            sim_require_nnan=True, nc=nc))

    devices = jax.devices()[:NCORES]
    mesh = Mesh(_np.asarray(devices), ("core",))
    donate = tuple(range(n_params, n_params + n_outs))
    sharded = jax.jit(
        shard_map(_body, mesh=mesh,
                  in_specs=(PartitionSpec("core"),) * (n_params + n_outs),
                  out_specs=(PartitionSpec("core"),) * n_outs,
                  check_rep=False),
        donate_argnums=donate, keep_unused=True)

    def run(in_maps):
        per_core = [[_np.asarray(m[name]) for name in in_names] for m in in_maps]
        concat_in = [_np.concatenate([per_core[c][i] for c in range(NCORES)], axis=0)
                     for i in range(n_params)]
        concat_zero = [_np.zeros((NCORES * z.shape[0], *z.shape[1:]), z.dtype)
                       for z in zero_outs]
        outs = sharded(*concat_in, *concat_zero)
        outs = [_np.asarray(o) for o in outs]
        return {name: outs[i] for i, name in enumerate(out_names)}

    return run


def _host_pp8(pos):
    """pos [BL?, N] f32 -> pp8 [.., 8, N] fp16 rows [qh,qh,qlr,ph,ph,pl,1,1]"""
    p = np.asarray(pos, np.float32)
    ph = p.astype(np.float16).astype(np.float32)
    pl = ((p - ph) * 4096.0).astype(np.float16)
    qq = ph * ph
    qh = qq.astype(np.float16).astype(np.float32)
    qlr = ((qq - qh) * 2048.0 +
           ph * pl.astype(np.float32)).astype(np.float16)
    ones = np.ones_like(p, np.float16)
    return np.stack([qh.astype(np.float16), qh.astype(np.float16), qlr,
                     ph.astype(np.float16), ph.astype(np.float16), pl,
                     ones, ones], axis=-2)


def _host_embt(embb):
    """emb bf16 [B?, N, D] -> embT bf16 [B?, 2, 128, N]"""
    return np.ascontiguousarray(
        embb.transpose(0, 2, 1).reshape(embb.shape[0], 2, 128, N))


def kernel(**inputs):
    import ml_dtypes
    emb = np.ascontiguousarray(inputs["embeddings"], dtype=np.float32)
    pos = np.ascontiguousarray(inputs["positions"], dtype=np.float32)
    grid = np.asarray(inputs["grid_points"], np.float64)[0, :, 0]
    params = dict(
        sigma=float(np.asarray(inputs["sigma"])),
        alpha=float(np.asarray(inputs["alpha"])),
        grid=grid,
        W_int=np.asarray(inputs["W_int"], np.float64),
        b_int=np.asarray(inputs["b_int"], np.float64),
        W_out=np.asarray(inputs["W_out"], np.float64),
        b_out=np.asarray(inputs["b_out"], np.float64),
        ln1_g=np.asarray(inputs["ln1_g"], np.float64),
        ln1_b=np.asarray(inputs["ln1_b"], np.float64),
        ln2_g=np.asarray(inputs["ln2_g"], np.float64),
        ln2_b=np.asarray(inputs["ln2_b"], np.float64),
    )
    key = hashlib.sha256(b"".join(np.asarray(v).tobytes() for v in params.values())).hexdigest()
    if key not in _CACHE:
        consts, flags = _host_plan(**params)
        nc = _build_module(flags)
        _CACHE[key] = (_make_runner(nc), consts)
    run, consts = _CACHE[key]

    embb = emb.astype(ml_dtypes.bfloat16)
    embt = _host_embt(embb)
    pp8 = _host_pp8(pos[..., 0])                 # [B, 8, N]
    in_maps = []
    for c in range(NCORES):
        m = {"emb": embb[BL * c:BL * (c + 1)],
             "embT": embt[BL * c:BL * (c + 1)],
             "pp8": pp8[BL * c:BL * (c + 1)]}
        m.update(consts)
        in_maps.append(m)
    outs = run(in_maps)
    return np.ascontiguousarray(outs["out"].astype(np.float32))
